# revision 2
# baseline (speedup 1.0000x reference)
"""AttentiveStatsPool Trainium2 Bass kernel (v5).

Full-input contract: kernel(**inputs) takes the unsharded numpy inputs and
returns the full (B, 2C, 1) output.  Internally shards the batch (B=16)
across 8 NeuronCores (2 samples per core), weights replicated, no cross-core
communication.

Math per sample (mask is all-ones per the problem spec):
  mean0/var0 over T per channel, std0 = sqrt(max(var0, 1e-5))
  m1 = w1[:, :C] @ x            (H, T)
  cH = w1[:, C:2C] @ mean0 + w1[:, 2C:] @ std0 + b1   (H,)
  r = relu(m1 + cH)
  LN over H: h = tanh(g1 * (r - mu)*rsqrt(var+1e-5) + be1)
  z = w2 @ h                    (b2 drops out: softmax over T is shift-inv)
  u = exp(z), Z = sum_t u, M1 = sum_t u*x, M2 = sum_t u*x^2
  mean = M1/Z, std = sqrt(max(M2/Z - mean^2, 1e-5))
  out = LayerNorm_{3072}(concat(mean, std)) * g2 + be2

v5 engine strategy (from per-op HW microbenchmarks):
  - accumulating DVE ops are 1x (~2.24us/chunk); plain TT 2x (1.2us);
    ACT always 1x (1.96us + 0.28us accum read); Pool TT ~5.8us, no accum
  - pass1: bn_stats x4 + bn_aggr (one DVE pass -> mean AND var, 2.9us/chunk);
    a few chunks ride ACT (Copy+acc, Square+acc) while ACT is idle early
  - pass2: exp+accZ on ACT; M1 via DVE STT+acc; M2 split across DVE STT /
    Pool TT + ACT Copy+acc (3-engine balance)
  - schedule: sample-1 pass-1 interleaved into sample-0 pass-2; LN chains
    overlapped with neighbouring phases; per-sample finals
"""

import numpy as np
import ml_dtypes

B, C, T, H = 16, 1536, 2000, 128
NCORES = 8
BLOC = B // NCORES          # 2 samples per core
KC = C // 128               # 12 channel chunks
QOFF = [0, 512, 1024, 1536]  # psum quarter offsets (512/512/512/464 -> flat 0:2000)
QLEN = [512, 512, 512, 464]
EPS = 1e-5
NB = BLOC * KC              # 24 accum columns, col = b*KC + k

# --- engine-placement knobs (tuned against trace) ---
PASS1_ACT = {(0, 1), (0, 4), (0, 7), (0, 10), (1, 1), (1, 5), (1, 9)}
M2_POOL = {(0, k) for k in range(12)} | {(1, 0), (1, 1), (1, 2)}
M2_SPLIT = set()            # M2 as DVE TT + ACT Copy+acc
M1_SPLIT = set()            # M1 as DVE TT + ACT Copy+acc
USE_POW = False             # rsqrt/sqrt via DVE pow instead of ACT Ln/Exp

_compiled = {}


# ---------------------------------------------------------------------------
# Workaround for walrus codegen 'Too many sync wait commands': this container's
# walrus supports only ONE sync-wait slot per instruction, but Tile's wait
# assignment can attach several.  Post-pass: move excess waits onto standalone
# InstNoOp carriers spliced immediately before the instruction on the same
# engine (same-engine program order makes this equivalent).
# ---------------------------------------------------------------------------

def _apply_tile_patch():
    import concourse.mybir as mybir
    import concourse.tile as tile
    from concourse.vector_clock import ScopedClock

    if getattr(tile.TileContext, "_wait_split_patched", False):
        return

    MAX_WAITS = 1

    def split_excess_waits(nc):
        for fn in nc.m.functions:
            for bb in fn.blocks:
                il = bb.instructions
                out = []
                changed = False
                for inst in il:
                    si = getattr(inst, "sync_info", None)
                    waits = list(si.on_wait) if si is not None else []
                    if len(waits) > MAX_WAITS:
                        for j, w in enumerate(waits[MAX_WAITS:]):
                            nop = mybir.InstNoOp(
                                name=f"{inst.name}-wsplit{j}",
                                sync_info=mybir.SyncInfo(on_wait=[w], on_update=[]),
                                bass_nofuse=True,
                                engine=inst.engine,
                            )
                            nc.register_instruction(nop, overwrite=True)
                            out.append(nop)
                        si.on_wait = waits[:MAX_WAITS]
                        changed = True
                    out.append(inst)
                if changed:
                    bb.instructions = out

    def _patched_drain_and_barrier(self, tick_clock, wait_clock):
        nc = self.nc
        drain_inst = nc.sync.drain()
        wait_clock.add_sem_waits(
            drain_inst.ins, ScopedClock({None: tick_clock.global_clock})
        )
        nc.all_engine_barrier()
        assert self.sems is not None
        popped = nc._tile_sem_poison_stack.pop()
        assert popped is self._sem_poison
        nc.clear_and_free_semaphores(list(self.sems.allocated().values()))
        nc.all_engine_barrier()
        split_excess_waits(nc)

    tile.TileContext._drain_and_barrier = _patched_drain_and_barrier
    tile.TileContext._wait_split_patched = True


# ---------------------------------------------------------------------------
# Device kernel builder (one NeuronCore, BLOC samples)
# ---------------------------------------------------------------------------

def _build():
    import concourse.bass as bass
    import concourse.tile as tile
    import concourse.mybir as mybir
    from contextlib import ExitStack

    _apply_tile_patch()

    f32 = mybir.dt.float32
    bf16 = mybir.dt.bfloat16
    AL = mybir.AluOpType
    AF = mybir.ActivationFunctionType

    nc = bass.Bass(name="attnpool")

    xd = nc.dram_tensor("x", [BLOC, KC, 128, T], bf16, kind="ExternalInput")
    wad = nc.dram_tensor("wa", [128, KC, 128], bf16, kind="ExternalInput")
    wbcd = nc.dram_tensor("wbc", [128, 2 * KC, 128], bf16, kind="ExternalInput")
    w2td = nc.dram_tensor("w2t", [128, KC, 128], bf16, kind="ExternalInput")
    onesHd = nc.dram_tensor("onesH", [128, 128], bf16, kind="ExternalInput")
    onesfd = nc.dram_tensor("ones_f", [128, 128], f32, kind="ExternalInput")
    b1d = nc.dram_tensor("b1v", [128, 1], f32, kind="ExternalInput")
    g1d = nc.dram_tensor("g1v", [128, 1], f32, kind="ExternalInput")
    be1d = nc.dram_tensor("be1v", [128, 1], f32, kind="ExternalInput")
    g2d = nc.dram_tensor("g2v", [128, 2 * KC], f32, kind="ExternalInput")
    be2d = nc.dram_tensor("be2v", [128, 2 * KC], f32, kind="ExternalInput")
    yd = nc.dram_tensor("y", [BLOC, 128, 2 * KC], f32, kind="ExternalOutput")

    with tile.TileContext(nc) as tc, ExitStack() as ctx:
        singles = ctx.enter_context(tc.tile_pool(name="singles", bufs=1))
        xpool = ctx.enter_context(tc.tile_pool(name="xcache", bufs=1))
        work = ctx.enter_context(tc.tile_pool(name="work", bufs=1))
        dscr = ctx.enter_context(tc.tile_pool(name="dscr", bufs=3))
        bnp = ctx.enter_context(tc.tile_pool(name="bnp", bufs=2))
        pA = ctx.enter_context(tc.tile_pool(name="pA", bufs=1, space="PSUM"))

        # ---- small early-needed weights first (wa gates m1), then x loads ----
        wa_sb = singles.tile([128, KC, 128], bf16)
        nc.sync.dma_start(out=wa_sb, in_=wad[:, :, :])
        onesH_sb = singles.tile([128, 128], bf16)          # value 1/H
        nc.sync.dma_start(out=onesH_sb, in_=onesHd[:, :])
        b1_sb = singles.tile([128, 1], f32)
        nc.sync.dma_start(out=b1_sb, in_=b1d[:, :])
        g1_sb = singles.tile([128, 1], f32)
        nc.sync.dma_start(out=g1_sb, in_=g1d[:, :])
        be1_sb = singles.tile([128, 1], f32)
        nc.sync.dma_start(out=be1_sb, in_=be1d[:, :])

        x_bf = xpool.tile([128, BLOC, KC, T], bf16)         # 96 KB/part
        for b in range(BLOC):
            for k in range(KC):
                nc.sync.dma_start(out=x_bf[:, b, k, :], in_=xd[b, k, :, :])

        # ---- remaining weights / constants ----
        wbc_sb = singles.tile([128, 2 * KC, 128], bf16)
        nc.sync.dma_start(out=wbc_sb, in_=wbcd[:, :, :])
        w2t_sb = singles.tile([128, KC, 128], bf16)
        nc.sync.dma_start(out=w2t_sb, in_=w2td[:, :, :])
        onesf_sb = singles.tile([128, 128], f32)
        nc.sync.dma_start(out=onesf_sb, in_=onesfd[:, :])
        g2_sb = singles.tile([128, 2 * KC], f32)
        nc.sync.dma_start(out=g2_sb, in_=g2d[:, :])
        be2_sb = singles.tile([128, 2 * KC], f32)
        nc.sync.dma_start(out=be2_sb, in_=be2d[:, :])

        eps_sb = singles.tile([128, 1], f32)
        nc.vector.memset(eps_sb, EPS)

        # ---- persistent SBUF state ----
        h_bf = work.tile([128, BLOC, T], bf16)              # attention hidden
        sumx = work.tile([128, NB], f32)                    # ACT-path pass1 sums
        sumx2 = work.tile([128, NB], f32)
        bnag = work.tile([128, NB, 2], f32)                 # (mean, var) per chunk
        accZ = work.tile([128, NB], f32)
        accM1 = work.tile([128, NB], f32)
        accM2 = work.tile([128, NB], f32)
        mv_bf = work.tile([128, BLOC, 2 * KC], bf16)        # [mean0 | std0] bf16
        biasv = work.tile([128, BLOC], f32)
        # LN scratch (reused across samples)
        r_bf = work.tile([128, T], bf16)
        d_bf = work.tile([128, T], bf16)
        d2_bf = work.tile([128, T], bf16)
        rs_bf = work.tile([128, T], bf16)
        # reduction dump buffers (one per engine to avoid cross-engine WAW)
        sdump_a = work.tile([128, T], bf16)
        # pass1 finalize scratch
        msq = work.tile([128, NB], f32)
        var0 = work.tile([128, NB], f32)
        std0f = work.tile([128, NB], f32)

        def emit_pass1_chunk(b, k, m1ps):
            col = b * KC + k
            xc = x_bf[:, b, k, :]
            for q in range(4):
                o, ln = QOFF[q], QLEN[q]
                nc.tensor.matmul(
                    m1ps[:, o:o + ln], wa_sb[:, k, :],
                    x_bf[:, b, k, o:o + ln],
                    start=(k == 0), stop=(k == KC - 1),
                )
            if (b, k) in PASS1_ACT:
                # ACT path: sum(x) and sum(x^2), tiny DVE converts to mean/var
                nc.scalar.activation(
                    out=sdump_a, in_=xc, func=AF.Copy,
                    accum_out=sumx[:, col:col + 1],
                )
                nc.scalar.activation(
                    out=sdump_a, in_=xc, func=AF.Square,
                    accum_out=sumx2[:, col:col + 1],
                )
                nc.vector.tensor_scalar(
                    out=bnag[:, col, 0:1], in0=sumx[:, col:col + 1],
                    scalar1=1.0 / T, scalar2=None, op0=AL.mult,
                )
                nc.vector.tensor_mul(
                    out=msq[:, col:col + 1], in0=bnag[:, col, 0:1],
                    in1=bnag[:, col, 0:1],
                )
                nc.vector.scalar_tensor_tensor(
                    out=bnag[:, col, 1:2], in0=sumx2[:, col:col + 1],
                    scalar=1.0 / T, in1=msq[:, col:col + 1],
                    op0=AL.mult, op1=AL.subtract,
                )
            else:
                # DVE path: bn_stats gives mean AND var in one pass
                bns = bnp.tile([128, 4, 6], f32, tag="bns", name=f"bns{col}")
                for q in range(4):
                    nc.vector.bn_stats(
                        out=bns[:, q, :], in_=xc[:, 500 * q:500 * (q + 1)]
                    )
                nc.vector.bn_aggr(out=bnag[:, col, :], in_=bns)

        def emit_bn_finalize(b):
            bsl = slice(b * KC, (b + 1) * KC)
            # std0 = sqrt(max(var0, eps)); mean/std -> bf16 [mean0 | std0]
            nc.vector.tensor_scalar_max(
                out=var0[:, bsl], in0=bnag[:, bsl, 1], scalar1=EPS,
            )
            if USE_POW:
                nc.vector.tensor_scalar(
                    out=std0f[:, bsl], in0=var0[:, bsl],
                    scalar1=0.5, scalar2=None, op0=AL.pow,
                )
            else:
                nc.scalar.activation(out=std0f[:, bsl], in_=var0[:, bsl], func=AF.Ln)
                nc.scalar.activation(out=std0f[:, bsl], in_=std0f[:, bsl],
                                     func=AF.Exp, scale=0.5)
            nc.vector.tensor_copy(out=mv_bf[:, b, 0:KC], in_=bnag[:, bsl, 0])
            nc.vector.tensor_copy(out=mv_bf[:, b, KC:2 * KC], in_=std0f[:, bsl])

        def emit_ch(b, tag):
            chps = pA.tile([128, 1], f32, tag=tag, name=f"ch{b}")
            for j in range(2 * KC):
                nc.tensor.matmul(
                    chps, wbc_sb[:, j, :], mv_bf[:, b, j:j + 1],
                    start=(j == 0), stop=(j == 2 * KC - 1),
                )
            nc.vector.tensor_add(out=biasv[:, b:b + 1], in0=chps, in1=b1_sb)

        def emit_ln(b, m1ps, tag):
            """r = relu(m1+bias); h = tanh(g1*(r-mu)*rsqrt(var+eps)+be1)."""
            nc.scalar.activation(
                out=r_bf, in_=m1ps[:, 0:T], func=AF.Relu,
                bias=biasv[:, b:b + 1], scale=1.0,
            )
            mups = pA.tile([128, 2048], f32, tag=tag, name=f"mups{b}")
            for q in range(4):
                o, ln = QOFF[q], QLEN[q]
                nc.tensor.matmul(
                    mups[:, o:o + ln], onesH_sb, r_bf[:, o:o + ln],
                    start=True, stop=True,
                )
            # d = r - mu  (STT reads mu straight from psum; no evac copy)
            nc.vector.scalar_tensor_tensor(
                out=d_bf, in0=r_bf, scalar=1.0, in1=mups[:, 0:T],
                op0=AL.mult, op1=AL.subtract,
            )
            nc.vector.tensor_mul(out=d2_bf, in0=d_bf, in1=d_bf)
            varps = pA.tile([128, 2048], f32, tag=tag, name=f"varps{b}")
            for q in range(4):
                o, ln = QOFF[q], QLEN[q]
                nc.tensor.matmul(
                    varps[:, o:o + ln], onesH_sb, d2_bf[:, o:o + ln],
                    start=True, stop=True,
                )
            if USE_POW:
                # rs = (var + eps)^-0.5 on DVE
                nc.vector.tensor_scalar(
                    out=rs_bf, in0=varps[:, 0:T], scalar1=EPS, scalar2=-0.5,
                    op0=AL.add, op1=AL.pow,
                )
            else:
                # rsqrt(var+eps) = exp(-0.5*ln(var+eps))
                nc.scalar.activation(
                    out=rs_bf, in_=varps[:, 0:T], func=AF.Ln,
                    bias=eps_sb, scale=1.0,
                )
                nc.scalar.activation(
                    out=rs_bf, in_=rs_bf, func=AF.Exp, scale=-0.5,
                )
            nc.vector.tensor_mul(out=d_bf, in0=d_bf, in1=rs_bf)
            nc.scalar.activation(
                out=h_bf[:, b, :], in_=d_bf, func=AF.Tanh,
                bias=be1_sb, scale=g1_sb,
            )

        def emit_pass2_chunk(b, k, tag):
            col = b * KC + k
            xc = x_bf[:, b, k, :]
            zps = pA.tile([128, 2048], f32, tag=tag, name=f"z{b}_{k}")
            for q in range(4):
                o, ln = QOFF[q], QLEN[q]
                nc.tensor.matmul(
                    zps[:, o:o + ln], w2t_sb[:, k, :],
                    h_bf[:, b, o:o + ln],
                    start=True, stop=True,
                )
            u_bf = dscr.tile([128, T], bf16, tag="u")
            nc.scalar.activation(
                out=u_bf, in_=zps[:, 0:T], func=AF.Exp,
                accum_out=accZ[:, col:col + 1],
            )
            p_bf = dscr.tile([128, T], bf16, tag="p")
            if (b, k) in M1_SPLIT:
                nc.vector.tensor_mul(out=p_bf, in0=u_bf, in1=xc)
                nc.scalar.activation(
                    out=sdump_a, in_=p_bf, func=AF.Copy,
                    accum_out=accM1[:, col:col + 1],
                )
            else:
                nc.vector.scalar_tensor_tensor(
                    out=p_bf, in0=u_bf, scalar=1.0, in1=xc,
                    op0=AL.mult, op1=AL.mult,
                    accum_out=accM1[:, col:col + 1],
                )
            q_bf = dscr.tile([128, T], bf16, tag="q")
            if (b, k) in M2_POOL:
                nc.gpsimd.tensor_mul(out=q_bf, in0=p_bf, in1=xc)
                nc.scalar.activation(
                    out=sdump_a, in_=q_bf, func=AF.Copy,
                    accum_out=accM2[:, col:col + 1],
                )
            elif (b, k) in M2_SPLIT:
                nc.vector.tensor_mul(out=q_bf, in0=p_bf, in1=xc)
                nc.scalar.activation(
                    out=sdump_a, in_=q_bf, func=AF.Copy,
                    accum_out=accM2[:, col:col + 1],
                )
            else:
                nc.vector.scalar_tensor_tensor(
                    out=q_bf, in0=p_bf, scalar=1.0, in1=xc,
                    op0=AL.mult, op1=AL.mult,
                    accum_out=accM2[:, col:col + 1],
                )

        def emit_final(b):
            """pooled mean/std -> LayerNorm(3072) -> DMA out, for sample b."""
            bsl = slice(b * KC, (b + 1) * KC)
            zr = work.tile([128, KC], f32, tag="zr", name="zr")
            nc.vector.reciprocal(out=zr, in_=accZ[:, bsl])
            v = work.tile([128, 2 * KC], f32, tag="vfin", name="vfin")
            nc.vector.tensor_mul(out=v[:, 0:KC], in0=accM1[:, bsl], in1=zr)
            ve2 = work.tile([128, KC], f32, tag="ve2", name="ve2")
            nc.vector.tensor_mul(out=ve2, in0=accM2[:, bsl], in1=zr)
            vmsq = work.tile([128, KC], f32, tag="vmsq", name="vmsq")
            nc.vector.tensor_mul(out=vmsq, in0=v[:, 0:KC], in1=v[:, 0:KC])
            nc.vector.tensor_sub(out=ve2, in0=ve2, in1=vmsq)
            nc.vector.tensor_scalar_max(out=ve2, in0=ve2, scalar1=EPS)
            if USE_POW:
                nc.vector.tensor_scalar(
                    out=v[:, KC:2 * KC], in0=ve2, scalar1=0.5, scalar2=None,
                    op0=AL.pow,
                )
            else:
                nc.scalar.activation(out=v[:, KC:2 * KC], in_=ve2, func=AF.Ln)
                nc.scalar.activation(out=v[:, KC:2 * KC], in_=v[:, KC:2 * KC],
                                     func=AF.Exp, scale=0.5)

            v2 = work.tile([128, 2 * KC], f32, tag="v2fin", name="v2fin")
            nc.vector.tensor_mul(out=v2, in0=v, in1=v)
            svp = pA.tile([128, 2 * KC], f32, tag="A", name="sv")
            nc.tensor.matmul(svp, onesf_sb, v, start=True, stop=True)
            sv2p = pA.tile([128, 2 * KC], f32, tag="A1", name="sv2")
            nc.tensor.matmul(sv2p, onesf_sb, v2, start=True, stop=True)
            muf = work.tile([128, 1], f32, tag="muf", name="muf")
            nc.vector.tensor_reduce(
                out=muf, in_=svp, axis=mybir.AxisListType.X, op=AL.add
            )
            s2r = work.tile([128, 1], f32, tag="s2r", name="s2r")
            nc.vector.tensor_reduce(
                out=s2r, in_=sv2p, axis=mybir.AxisListType.X, op=AL.add
            )
            nc.vector.tensor_scalar_mul(out=muf, in0=muf, scalar1=1.0 / (2 * C))
            musq = work.tile([128, 1], f32, tag="musq", name="musq")
            nc.vector.tensor_mul(out=musq, in0=muf, in1=muf)
            nc.vector.scalar_tensor_tensor(
                out=s2r, in0=s2r, scalar=1.0 / (2 * C), in1=musq,
                op0=AL.mult, op1=AL.subtract,
            )
            if USE_POW:
                nc.vector.tensor_scalar(
                    out=s2r, in0=s2r, scalar1=EPS, scalar2=-0.5,
                    op0=AL.add, op1=AL.pow,
                )
            else:
                nc.scalar.activation(
                    out=s2r, in_=s2r, func=AF.Ln, bias=eps_sb, scale=1.0
                )
                nc.scalar.activation(out=s2r, in_=s2r, func=AF.Exp, scale=-0.5)
            vout = work.tile([128, 2 * KC], f32, tag="vout", name="vout")
            nc.vector.tensor_scalar(
                out=vout, in0=v, scalar1=muf, scalar2=s2r,
                op0=AL.subtract, op1=AL.mult,
            )
            nc.vector.tensor_mul(out=vout, in0=vout, in1=g2_sb)
            nc.vector.tensor_add(out=vout, in0=vout, in1=be2_sb)
            nc.sync.dma_start(out=yd[b, :, :], in_=vout)

        # ================= schedule =================
        # Two 4-bank psum regions (tags A / A1); all tiles rotate within them.
        # A(b0)
        m1ps0 = pA.tile([128, 2048], f32, tag="A", name="m1ps0")
        for k in range(KC):
            emit_pass1_chunk(0, k, m1ps0)
        emit_bn_finalize(0)
        emit_ch(0, "A1")
        # pre-MID: a few b1 pass-1 chunks keep DVE/ACT fed during LN(b0)
        m1ps1 = pA.tile([128, 2048], f32, tag="A1", name="m1ps1")
        emit_pass1_chunk(1, 0, m1ps1)
        emit_pass1_chunk(1, 1, m1ps1)
        emit_ln(0, m1ps0, "A")
        emit_pass1_chunk(1, 2, m1ps1)
        # MID: pass2(b0) interleaved with the rest of A(b1)
        for k in range(9):
            emit_pass2_chunk(0, k, "A")
            emit_pass1_chunk(1, k + 3, m1ps1)
        emit_bn_finalize(1)
        emit_ch(1, "A")
        emit_pass2_chunk(0, 9, "A")
        emit_pass2_chunk(0, 10, "A")
        emit_ln(1, m1ps1, "A1")       # overlaps pass2(b0) tail in region A1
        emit_pass2_chunk(0, 11, "A")
        emit_final(0)
        # TAIL: pass2(b1), zps double-buffered across both regions
        for k in range(KC):
            emit_pass2_chunk(1, k, "A" if k % 2 == 0 else "A1")
        emit_final(1)

    return nc


def _get_nc():
    if "nc" not in _compiled:
        _compiled["nc"] = _build()
    return _compiled["nc"]


def _prep_common(w1, b1, g1, be1, w2, g2, be2):
    bf = ml_dtypes.bfloat16
    # SBUF-layout weights (partition-major, contiguous DMA):
    # wa[c, k, h] = w1[h, 128k+c] ; wbc[c, j, h] ; w2t[h, k, c] = w2[128k+c, h]
    w1 = np.asarray(w1, np.float32)
    w1a = np.ascontiguousarray(
        w1[:, :C].T.reshape(KC, 128, H).transpose(1, 0, 2)).astype(bf)
    w1bT = w1[:, C:2 * C].T.reshape(KC, 128, H)
    w1cT = w1[:, 2 * C:].T.reshape(KC, 128, H)
    wbc = np.ascontiguousarray(
        np.concatenate([w1bT, w1cT], axis=0).transpose(1, 0, 2)
    ).astype(bf)
    w2t = np.ascontiguousarray(
        np.asarray(w2, np.float32).reshape(KC, 128, H).transpose(2, 0, 1)
    ).astype(bf)

    return {
        "wa": w1a,
        "wbc": wbc,
        "w2t": w2t,
        "onesH": np.full((128, 128), 1.0 / H, dtype=bf),
        "ones_f": np.ones((128, 128), dtype=np.float32),
        "b1v": np.asarray(b1, np.float32).reshape(128, 1),
        "g1v": np.asarray(g1, np.float32).reshape(128, 1),
        "be1v": np.asarray(be1, np.float32).reshape(128, 1),
        "g2v": np.ascontiguousarray(
            np.asarray(g2, np.float32).reshape(2 * KC, 128).T),
        "be2v": np.ascontiguousarray(np.asarray(be2, np.float32).reshape(2 * KC, 128).T),
    }


def kernel(x, mask, w1, b1, g1, be1, w2, b2, g2, be2, _trace=False, _tmpdir=None):
    from concourse.bass_utils import run_bass_kernel_spmd

    bf = ml_dtypes.bfloat16
    x = np.asarray(x, dtype=np.float32)
    common = _prep_common(w1, b1, g1, be1, w2, g2, be2)

    in_maps = []
    for i in range(NCORES):
        xi = np.ascontiguousarray(
            x[i * BLOC:(i + 1) * BLOC].reshape(BLOC, KC, 128, T)
        ).astype(bf)
        in_maps.append({"x": xi, **common})

    nc = _get_nc()
    kwargs = {}
    if _trace:
        kwargs = {"trace": True, "tmpdir": _tmpdir}
    res = run_bass_kernel_spmd(nc, in_maps, core_ids=list(range(NCORES)), **kwargs)

    out = np.empty((B, 2 * C, 1), dtype=np.float32)
    for i in range(NCORES):
        # y[b, p, k] -> channel 128k+p
        yi = res.results[i]["y"].transpose(0, 2, 1).reshape(BLOC, 2 * C)
        out[i * BLOC:(i + 1) * BLOC, :, 0] = yi
    if _trace:
        return out, res
    return out


# revision 5
# speedup vs baseline: 1.1421x; 1.1421x over previous
"""AttentiveStatsPool Trainium2 Bass kernel (v5).

Full-input contract: kernel(**inputs) takes the unsharded numpy inputs and
returns the full (B, 2C, 1) output.  Internally shards the batch (B=16)
across 8 NeuronCores (2 samples per core), weights replicated, no cross-core
communication.

Math per sample (mask is all-ones per the problem spec):
  mean0/var0 over T per channel, std0 = sqrt(max(var0, 1e-5))
  m1 = w1[:, :C] @ x            (H, T)
  cH = w1[:, C:2C] @ mean0 + w1[:, 2C:] @ std0 + b1   (H,)
  r = relu(m1 + cH)
  LN over H: h = tanh(g1 * (r - mu)*rsqrt(var+1e-5) + be1)
  z = w2 @ h                    (b2 drops out: softmax over T is shift-inv)
  u = exp(z), Z = sum_t u, M1 = sum_t u*x, M2 = sum_t u*x^2
  mean = M1/Z, std = sqrt(max(M2/Z - mean^2, 1e-5))
  out = LayerNorm_{3072}(concat(mean, std)) * g2 + be2

v5 engine strategy (from per-op HW microbenchmarks):
  - accumulating DVE ops are 1x (~2.24us/chunk); plain TT 2x (1.2us);
    ACT always 1x (1.96us + 0.28us accum read); Pool TT ~5.8us, no accum
  - pass1: bn_stats x4 + bn_aggr (one DVE pass -> mean AND var, 2.9us/chunk);
    a few chunks ride ACT (Copy+acc, Square+acc) while ACT is idle early
  - pass2: exp+accZ on ACT; M1 via DVE STT+acc; M2 split across DVE STT /
    Pool TT + ACT Copy+acc (3-engine balance)
  - schedule: sample-1 pass-1 interleaved into sample-0 pass-2; LN chains
    overlapped with neighbouring phases; per-sample finals
"""

import numpy as np
import ml_dtypes

B, C, T, H = 16, 1536, 2000, 128
NCORES = 8
BLOC = B // NCORES          # 2 samples per core
KC = C // 128               # 12 channel chunks
QOFF = [0, 512, 1024, 1536]  # psum quarter offsets (512/512/512/464 -> flat 0:2000)
QLEN = [512, 512, 512, 464]
EPS = 1e-5
NB = BLOC * KC              # 24 accum columns, col = b*KC + k

# --- engine-placement knobs (tuned against trace) ---
PASS1_ACT = {(0, 1), (0, 4), (0, 7), (0, 10), (1, 1), (1, 5), (1, 9)}
M2_POOL = set()             # pool TT causes SBUF contention: net negative
M2_SPLIT = ({(0, k) for k in range(12)} |
            {(1, k) for k in range(8)})   # DVE TT + ACT Copy+acc
M1_SPLIT = set()            # M1 as DVE TT + ACT Copy+acc
USE_POW = False             # DVE pow rejected by walrus ISA check

_compiled = {}


# ---------------------------------------------------------------------------
# Workaround for walrus codegen 'Too many sync wait commands': this container's
# walrus supports only ONE sync-wait slot per instruction, but Tile's wait
# assignment can attach several.  Post-pass: move excess waits onto standalone
# InstNoOp carriers spliced immediately before the instruction on the same
# engine (same-engine program order makes this equivalent).
# ---------------------------------------------------------------------------

def _apply_tile_patch():
    import concourse.mybir as mybir
    import concourse.tile as tile
    from concourse.vector_clock import ScopedClock

    if getattr(tile.TileContext, "_wait_split_patched", False):
        return

    MAX_WAITS = 1

    def split_excess_waits(nc):
        for fn in nc.m.functions:
            for bb in fn.blocks:
                il = bb.instructions
                out = []
                changed = False
                for inst in il:
                    si = getattr(inst, "sync_info", None)
                    waits = list(si.on_wait) if si is not None else []
                    if len(waits) > MAX_WAITS:
                        for j, w in enumerate(waits[MAX_WAITS:]):
                            nop = mybir.InstNoOp(
                                name=f"{inst.name}-wsplit{j}",
                                sync_info=mybir.SyncInfo(on_wait=[w], on_update=[]),
                                bass_nofuse=True,
                                engine=inst.engine,
                            )
                            nc.register_instruction(nop, overwrite=True)
                            out.append(nop)
                        si.on_wait = waits[:MAX_WAITS]
                        changed = True
                    out.append(inst)
                if changed:
                    bb.instructions = out

    def _patched_drain_and_barrier(self, tick_clock, wait_clock):
        nc = self.nc
        drain_inst = nc.sync.drain()
        wait_clock.add_sem_waits(
            drain_inst.ins, ScopedClock({None: tick_clock.global_clock})
        )
        nc.all_engine_barrier()
        assert self.sems is not None
        popped = nc._tile_sem_poison_stack.pop()
        assert popped is self._sem_poison
        nc.clear_and_free_semaphores(list(self.sems.allocated().values()))
        nc.all_engine_barrier()
        split_excess_waits(nc)

    tile.TileContext._drain_and_barrier = _patched_drain_and_barrier
    tile.TileContext._wait_split_patched = True


# ---------------------------------------------------------------------------
# Device kernel builder (one NeuronCore, BLOC samples)
# ---------------------------------------------------------------------------

def _build():
    import concourse.bass as bass
    import concourse.tile as tile
    import concourse.mybir as mybir
    from contextlib import ExitStack

    _apply_tile_patch()

    f32 = mybir.dt.float32
    bf16 = mybir.dt.bfloat16
    AL = mybir.AluOpType
    AF = mybir.ActivationFunctionType

    nc = bass.Bass(name="attnpool")

    xd = nc.dram_tensor("x", [BLOC, KC, 128, T], bf16, kind="ExternalInput")
    wad = nc.dram_tensor("wa", [128, KC, 128], bf16, kind="ExternalInput")
    wbcd = nc.dram_tensor("wbc", [128, 2 * KC, 128], bf16, kind="ExternalInput")
    w2td = nc.dram_tensor("w2t", [128, KC, 128], bf16, kind="ExternalInput")
    onesHd = nc.dram_tensor("onesH", [128, 128], bf16, kind="ExternalInput")
    onesfd = nc.dram_tensor("ones_f", [128, 128], f32, kind="ExternalInput")
    b1d = nc.dram_tensor("b1v", [128, 1], f32, kind="ExternalInput")
    g1d = nc.dram_tensor("g1v", [128, 1], f32, kind="ExternalInput")
    be1d = nc.dram_tensor("be1v", [128, 1], f32, kind="ExternalInput")
    g2d = nc.dram_tensor("g2v", [128, 2 * KC], f32, kind="ExternalInput")
    be2d = nc.dram_tensor("be2v", [128, 2 * KC], f32, kind="ExternalInput")
    yd = nc.dram_tensor("y", [BLOC, 128, 2 * KC], f32, kind="ExternalOutput")

    with tile.TileContext(nc) as tc, ExitStack() as ctx:
        singles = ctx.enter_context(tc.tile_pool(name="singles", bufs=1))
        xpool = ctx.enter_context(tc.tile_pool(name="xcache", bufs=1))
        work = ctx.enter_context(tc.tile_pool(name="work", bufs=1))
        dscr = ctx.enter_context(tc.tile_pool(name="dscr", bufs=3))
        bnp = ctx.enter_context(tc.tile_pool(name="bnp", bufs=2))
        pA = ctx.enter_context(tc.tile_pool(name="pA", bufs=1, space="PSUM"))

        # ---- small early-needed weights first (wa gates m1), then x loads ----
        wa_sb = singles.tile([128, KC, 128], bf16)
        nc.sync.dma_start(out=wa_sb, in_=wad[:, :, :])
        onesH_sb = singles.tile([128, 128], bf16)          # value 1/H
        nc.sync.dma_start(out=onesH_sb, in_=onesHd[:, :])
        b1_sb = singles.tile([128, 1], f32)
        nc.sync.dma_start(out=b1_sb, in_=b1d[:, :])
        g1_sb = singles.tile([128, 1], f32)
        nc.sync.dma_start(out=g1_sb, in_=g1d[:, :])
        be1_sb = singles.tile([128, 1], f32)
        nc.sync.dma_start(out=be1_sb, in_=be1d[:, :])

        x_bf = xpool.tile([128, BLOC, KC, T], bf16)         # 96 KB/part
        for b in range(BLOC):
            for k in range(KC):
                nc.sync.dma_start(out=x_bf[:, b, k, :], in_=xd[b, k, :, :])

        # ---- remaining weights / constants ----
        wbc_sb = singles.tile([128, 2 * KC, 128], bf16)
        nc.sync.dma_start(out=wbc_sb, in_=wbcd[:, :, :])
        w2t_sb = singles.tile([128, KC, 128], bf16)
        nc.sync.dma_start(out=w2t_sb, in_=w2td[:, :, :])
        onesf_sb = singles.tile([128, 128], f32)
        nc.sync.dma_start(out=onesf_sb, in_=onesfd[:, :])
        g2_sb = singles.tile([128, 2 * KC], f32)
        nc.sync.dma_start(out=g2_sb, in_=g2d[:, :])
        be2_sb = singles.tile([128, 2 * KC], f32)
        nc.sync.dma_start(out=be2_sb, in_=be2d[:, :])

        eps_sb = singles.tile([128, 1], f32)
        nc.vector.memset(eps_sb, EPS)

        # ---- persistent SBUF state ----
        h_bf = work.tile([128, BLOC, T], bf16)              # attention hidden
        sumx = work.tile([128, NB], f32)                    # ACT-path pass1 sums
        sumx2 = work.tile([128, NB], f32)
        bnag = work.tile([128, NB, 2], f32)                 # (mean, var) per chunk
        accZ = work.tile([128, NB], f32)
        accM1 = work.tile([128, NB], f32)
        accM2 = work.tile([128, NB], f32)
        mv_bf = work.tile([128, BLOC, 2 * KC], bf16)        # [mean0 | std0] bf16
        biasv = work.tile([128, BLOC], f32)
        # LN scratch (reused across samples)
        r_bf = work.tile([128, T], bf16)
        d_bf = work.tile([128, T], bf16)
        d2_bf = work.tile([128, T], bf16)
        rs_bf = work.tile([128, T], bf16)
        # reduction dump buffers (one per engine to avoid cross-engine WAW)
        sdump_a = work.tile([128, T], bf16)
        # pass1 finalize scratch
        msq = work.tile([128, NB], f32)
        var0 = work.tile([128, NB], f32)
        std0f = work.tile([128, NB], f32)

        def emit_pass1_chunk(b, k, m1ps):
            col = b * KC + k
            xc = x_bf[:, b, k, :]
            for q in range(4):
                o, ln = QOFF[q], QLEN[q]
                nc.tensor.matmul(
                    m1ps[:, o:o + ln], wa_sb[:, k, :],
                    x_bf[:, b, k, o:o + ln],
                    start=(k == 0), stop=(k == KC - 1),
                )
            if (b, k) in PASS1_ACT:
                # ACT path: sum(x) and sum(x^2), tiny DVE converts to mean/var
                nc.scalar.activation(
                    out=sdump_a, in_=xc, func=AF.Copy,
                    accum_out=sumx[:, col:col + 1],
                )
                nc.scalar.activation(
                    out=sdump_a, in_=xc, func=AF.Square,
                    accum_out=sumx2[:, col:col + 1],
                )
                nc.vector.tensor_scalar(
                    out=bnag[:, col, 0:1], in0=sumx[:, col:col + 1],
                    scalar1=1.0 / T, scalar2=None, op0=AL.mult,
                )
                nc.vector.tensor_mul(
                    out=msq[:, col:col + 1], in0=bnag[:, col, 0:1],
                    in1=bnag[:, col, 0:1],
                )
                nc.vector.scalar_tensor_tensor(
                    out=bnag[:, col, 1:2], in0=sumx2[:, col:col + 1],
                    scalar=1.0 / T, in1=msq[:, col:col + 1],
                    op0=AL.mult, op1=AL.subtract,
                )
            else:
                # DVE path: bn_stats gives mean AND var in one pass
                bns = bnp.tile([128, 4, 6], f32, tag="bns", name=f"bns{col}")
                for q in range(4):
                    nc.vector.bn_stats(
                        out=bns[:, q, :], in_=xc[:, 500 * q:500 * (q + 1)]
                    )
                nc.vector.bn_aggr(out=bnag[:, col, :], in_=bns)

        def emit_bn_finalize(b):
            bsl = slice(b * KC, (b + 1) * KC)
            # std0 = sqrt(max(var0, eps)); mean/std -> bf16 [mean0 | std0]
            nc.vector.tensor_scalar_max(
                out=var0[:, bsl], in0=bnag[:, bsl, 1], scalar1=EPS,
            )
            if USE_POW:
                nc.vector.tensor_scalar(
                    out=std0f[:, bsl], in0=var0[:, bsl],
                    scalar1=0.5, scalar2=None, op0=AL.pow,
                )
            else:
                nc.scalar.activation(out=std0f[:, bsl], in_=var0[:, bsl], func=AF.Ln)
                nc.scalar.activation(out=std0f[:, bsl], in_=std0f[:, bsl],
                                     func=AF.Exp, scale=0.5)
            nc.vector.tensor_copy(out=mv_bf[:, b, 0:KC], in_=bnag[:, bsl, 0])
            nc.vector.tensor_copy(out=mv_bf[:, b, KC:2 * KC], in_=std0f[:, bsl])

        def emit_ch(b, tag):
            chps = pA.tile([128, 1], f32, tag=tag, name=f"ch{b}")
            for j in range(2 * KC):
                nc.tensor.matmul(
                    chps, wbc_sb[:, j, :], mv_bf[:, b, j:j + 1],
                    start=(j == 0), stop=(j == 2 * KC - 1),
                )
            nc.vector.tensor_add(out=biasv[:, b:b + 1], in0=chps, in1=b1_sb)

        ln_state = {}

        def emit_ln_s1(b, m1ps, tag):
            """relu + column-mean matmuls."""
            nc.scalar.activation(
                out=r_bf, in_=m1ps[:, 0:T], func=AF.Relu,
                bias=biasv[:, b:b + 1], scale=1.0,
            )
            mups = pA.tile([128, 2048], f32, tag=tag, name=f"mups{b}")
            for q in range(4):
                o, ln = QOFF[q], QLEN[q]
                nc.tensor.matmul(
                    mups[:, o:o + ln], onesH_sb, r_bf[:, o:o + ln],
                    start=True, stop=True,
                )
            ln_state[b] = mups

        def emit_ln_s2(b, tag):
            """d = r - mu; d^2; column-var matmuls."""
            mups = ln_state[b]
            nc.vector.scalar_tensor_tensor(
                out=d_bf, in0=r_bf, scalar=1.0, in1=mups[:, 0:T],
                op0=AL.mult, op1=AL.subtract,
            )
            nc.vector.tensor_mul(out=d2_bf, in0=d_bf, in1=d_bf)
            varps = pA.tile([128, 2048], f32, tag=tag, name=f"varps{b}")
            for q in range(4):
                o, ln = QOFF[q], QLEN[q]
                nc.tensor.matmul(
                    varps[:, o:o + ln], onesH_sb, d2_bf[:, o:o + ln],
                    start=True, stop=True,
                )
            ln_state[b] = varps

        def emit_ln_s3(b):
            """rs = rsqrt(var+eps); h = tanh(g1*d*rs + be1)."""
            varps = ln_state.pop(b)
            nc.scalar.activation(
                out=rs_bf, in_=varps[:, 0:T], func=AF.Ln,
                bias=eps_sb, scale=1.0,
            )
            nc.scalar.activation(
                out=rs_bf, in_=rs_bf, func=AF.Exp, scale=-0.5,
            )
            nc.vector.tensor_mul(out=d_bf, in0=d_bf, in1=rs_bf)
            nc.scalar.activation(
                out=h_bf[:, b, :], in_=d_bf, func=AF.Tanh,
                bias=be1_sb, scale=g1_sb,
            )

        def emit_pass2_chunk(b, k, tag):
            col = b * KC + k
            xc = x_bf[:, b, k, :]
            zps = pA.tile([128, 2048], f32, tag=tag, name=f"z{b}_{k}")
            for q in range(4):
                o, ln = QOFF[q], QLEN[q]
                nc.tensor.matmul(
                    zps[:, o:o + ln], w2t_sb[:, k, :],
                    h_bf[:, b, o:o + ln],
                    start=True, stop=True,
                )
            u_bf = dscr.tile([128, T], bf16, tag="u")
            nc.scalar.activation(
                out=u_bf, in_=zps[:, 0:T], func=AF.Exp,
                accum_out=accZ[:, col:col + 1],
            )
            p_bf = dscr.tile([128, T], bf16, tag="p")
            if (b, k) in M1_SPLIT:
                nc.vector.tensor_mul(out=p_bf, in0=u_bf, in1=xc)
                nc.scalar.activation(
                    out=sdump_a, in_=p_bf, func=AF.Copy,
                    accum_out=accM1[:, col:col + 1],
                )
            else:
                nc.vector.scalar_tensor_tensor(
                    out=p_bf, in0=u_bf, scalar=1.0, in1=xc,
                    op0=AL.mult, op1=AL.mult,
                    accum_out=accM1[:, col:col + 1],
                )
            q_bf = dscr.tile([128, T], bf16, tag="q")
            if (b, k) in M2_POOL:
                nc.gpsimd.tensor_mul(out=q_bf, in0=p_bf, in1=xc)
                nc.scalar.activation(
                    out=sdump_a, in_=q_bf, func=AF.Copy,
                    accum_out=accM2[:, col:col + 1],
                )
            elif (b, k) in M2_SPLIT:
                nc.vector.tensor_mul(out=q_bf, in0=p_bf, in1=xc)
                nc.scalar.activation(
                    out=sdump_a, in_=q_bf, func=AF.Copy,
                    accum_out=accM2[:, col:col + 1],
                )
            else:
                nc.vector.scalar_tensor_tensor(
                    out=q_bf, in0=p_bf, scalar=1.0, in1=xc,
                    op0=AL.mult, op1=AL.mult,
                    accum_out=accM2[:, col:col + 1],
                )

        def emit_final(b):
            """pooled mean/std -> LayerNorm(3072) -> DMA out, for sample b."""
            bsl = slice(b * KC, (b + 1) * KC)
            zr = work.tile([128, KC], f32, tag="zr", name="zr")
            nc.vector.reciprocal(out=zr, in_=accZ[:, bsl])
            v = work.tile([128, 2 * KC], f32, tag="vfin", name="vfin")
            nc.vector.tensor_mul(out=v[:, 0:KC], in0=accM1[:, bsl], in1=zr)
            ve2 = work.tile([128, KC], f32, tag="ve2", name="ve2")
            nc.vector.tensor_mul(out=ve2, in0=accM2[:, bsl], in1=zr)
            vmsq = work.tile([128, KC], f32, tag="vmsq", name="vmsq")
            nc.vector.tensor_mul(out=vmsq, in0=v[:, 0:KC], in1=v[:, 0:KC])
            nc.vector.tensor_sub(out=ve2, in0=ve2, in1=vmsq)
            nc.vector.tensor_scalar_max(out=ve2, in0=ve2, scalar1=EPS)
            if USE_POW:
                nc.vector.tensor_scalar(
                    out=v[:, KC:2 * KC], in0=ve2, scalar1=0.5, scalar2=None,
                    op0=AL.pow,
                )
            else:
                nc.scalar.activation(out=v[:, KC:2 * KC], in_=ve2, func=AF.Ln)
                nc.scalar.activation(out=v[:, KC:2 * KC], in_=v[:, KC:2 * KC],
                                     func=AF.Exp, scale=0.5)

            v2 = work.tile([128, 2 * KC], f32, tag="v2fin", name="v2fin")
            nc.vector.tensor_mul(out=v2, in0=v, in1=v)
            svp = pA.tile([128, 2 * KC], f32, tag="A", name="sv")
            nc.tensor.matmul(svp, onesf_sb, v, start=True, stop=True)
            sv2p = pA.tile([128, 2 * KC], f32, tag="A1", name="sv2")
            nc.tensor.matmul(sv2p, onesf_sb, v2, start=True, stop=True)
            muf = work.tile([128, 1], f32, tag="muf", name="muf")
            nc.vector.tensor_reduce(
                out=muf, in_=svp, axis=mybir.AxisListType.X, op=AL.add
            )
            s2r = work.tile([128, 1], f32, tag="s2r", name="s2r")
            nc.vector.tensor_reduce(
                out=s2r, in_=sv2p, axis=mybir.AxisListType.X, op=AL.add
            )
            nc.vector.tensor_scalar_mul(out=muf, in0=muf, scalar1=1.0 / (2 * C))
            musq = work.tile([128, 1], f32, tag="musq", name="musq")
            nc.vector.tensor_mul(out=musq, in0=muf, in1=muf)
            nc.vector.scalar_tensor_tensor(
                out=s2r, in0=s2r, scalar=1.0 / (2 * C), in1=musq,
                op0=AL.mult, op1=AL.subtract,
            )
            if USE_POW:
                nc.vector.tensor_scalar(
                    out=s2r, in0=s2r, scalar1=EPS, scalar2=-0.5,
                    op0=AL.add, op1=AL.pow,
                )
            else:
                nc.scalar.activation(
                    out=s2r, in_=s2r, func=AF.Ln, bias=eps_sb, scale=1.0
                )
                nc.scalar.activation(out=s2r, in_=s2r, func=AF.Exp, scale=-0.5)
            vout = work.tile([128, 2 * KC], f32, tag="vout", name="vout")
            nc.vector.tensor_scalar(
                out=vout, in0=v, scalar1=muf, scalar2=s2r,
                op0=AL.subtract, op1=AL.mult,
            )
            nc.vector.tensor_mul(out=vout, in0=vout, in1=g2_sb)
            nc.vector.tensor_add(out=vout, in0=vout, in1=be2_sb)
            nc.sync.dma_start(out=yd[b, :, :], in_=vout)

        # ================= schedule =================
        # Two 4-bank psum regions (tags A / A1); all tiles rotate within them.
        # A(b0)
        m1ps0 = pA.tile([128, 2048], f32, tag="A", name="m1ps0")
        for k in range(KC):
            emit_pass1_chunk(0, k, m1ps0)
        emit_bn_finalize(0)
        emit_ch(0, "A1")
        # LN(b0) staged, with b1 pass-1 chunks as engine filler between stages
        m1ps1 = pA.tile([128, 2048], f32, tag="A1", name="m1ps1")
        emit_pass1_chunk(1, 0, m1ps1)
        emit_ln_s1(0, m1ps0, "A")
        emit_pass1_chunk(1, 1, m1ps1)   # ACT chunk fills relu->Ln gap
        emit_ln_s2(0, "A")
        emit_pass1_chunk(1, 2, m1ps1)   # bn chunk fills DVE while ACT Ln/Exp
        emit_ln_s3(0)
        # MID: pass2(b0) interleaved with the rest of A(b1)
        for k in range(9):
            emit_pass2_chunk(0, k, "A")
            emit_pass1_chunk(1, k + 3, m1ps1)
        emit_bn_finalize(1)
        emit_ch(1, "A")
        # LN(b1) staged, pass2(b0) tail chunks as filler
        emit_ln_s1(1, m1ps1, "A1")
        emit_pass2_chunk(0, 9, "A")
        emit_ln_s2(1, "A1")
        emit_pass2_chunk(0, 10, "A")
        emit_ln_s3(1)
        emit_pass2_chunk(0, 11, "A")
        emit_final(0)
        # TAIL: pass2(b1), zps double-buffered across both regions
        for k in range(KC):
            emit_pass2_chunk(1, k, "A" if k % 2 == 0 else "A1")
        emit_final(1)

    return nc


def _get_nc():
    if "nc" not in _compiled:
        _compiled["nc"] = _build()
    return _compiled["nc"]


def _prep_common(w1, b1, g1, be1, w2, g2, be2):
    bf = ml_dtypes.bfloat16
    # SBUF-layout weights (partition-major, contiguous DMA):
    # wa[c, k, h] = w1[h, 128k+c] ; wbc[c, j, h] ; w2t[h, k, c] = w2[128k+c, h]
    w1 = np.asarray(w1, np.float32)
    w1a = np.ascontiguousarray(
        w1[:, :C].T.reshape(KC, 128, H).transpose(1, 0, 2)).astype(bf)
    w1bT = w1[:, C:2 * C].T.reshape(KC, 128, H)
    w1cT = w1[:, 2 * C:].T.reshape(KC, 128, H)
    wbc = np.ascontiguousarray(
        np.concatenate([w1bT, w1cT], axis=0).transpose(1, 0, 2)
    ).astype(bf)
    w2t = np.ascontiguousarray(
        np.asarray(w2, np.float32).reshape(KC, 128, H).transpose(2, 0, 1)
    ).astype(bf)

    return {
        "wa": w1a,
        "wbc": wbc,
        "w2t": w2t,
        "onesH": np.full((128, 128), 1.0 / H, dtype=bf),
        "ones_f": np.ones((128, 128), dtype=np.float32),
        "b1v": np.asarray(b1, np.float32).reshape(128, 1),
        "g1v": np.asarray(g1, np.float32).reshape(128, 1),
        "be1v": np.asarray(be1, np.float32).reshape(128, 1),
        "g2v": np.ascontiguousarray(
            np.asarray(g2, np.float32).reshape(2 * KC, 128).T),
        "be2v": np.ascontiguousarray(np.asarray(be2, np.float32).reshape(2 * KC, 128).T),
    }


def kernel(x, mask, w1, b1, g1, be1, w2, b2, g2, be2, _trace=False, _tmpdir=None):
    from concourse.bass_utils import run_bass_kernel_spmd

    bf = ml_dtypes.bfloat16
    x = np.asarray(x, dtype=np.float32)
    common = _prep_common(w1, b1, g1, be1, w2, g2, be2)

    in_maps = []
    for i in range(NCORES):
        xi = np.ascontiguousarray(
            x[i * BLOC:(i + 1) * BLOC].reshape(BLOC, KC, 128, T)
        ).astype(bf)
        in_maps.append({"x": xi, **common})

    nc = _get_nc()
    kwargs = {}
    if _trace:
        kwargs = {"trace": True, "tmpdir": _tmpdir}
    res = run_bass_kernel_spmd(nc, in_maps, core_ids=list(range(NCORES)), **kwargs)

    out = np.empty((B, 2 * C, 1), dtype=np.float32)
    for i in range(NCORES):
        # y[b, p, k] -> channel 128k+p
        yi = res.results[i]["y"].transpose(0, 2, 1).reshape(BLOC, 2 * C)
        out[i * BLOC:(i + 1) * BLOC, :, 0] = yi
    if _trace:
        return out, res
    return out


# revision 11
# speedup vs baseline: 1.1650x; 1.0200x over previous
"""AttentiveStatsPool Trainium2 Bass kernel (v5).

Full-input contract: kernel(**inputs) takes the unsharded numpy inputs and
returns the full (B, 2C, 1) output.  Internally shards the batch (B=16)
across 8 NeuronCores (2 samples per core), weights replicated, no cross-core
communication.

Math per sample (mask is all-ones per the problem spec):
  mean0/var0 over T per channel, std0 = sqrt(max(var0, 1e-5))
  m1 = w1[:, :C] @ x            (H, T)
  cH = w1[:, C:2C] @ mean0 + w1[:, 2C:] @ std0 + b1   (H,)
  r = relu(m1 + cH)
  LN over H: h = tanh(g1 * (r - mu)*rsqrt(var+1e-5) + be1)
  z = w2 @ h                    (b2 drops out: softmax over T is shift-inv)
  u = exp(z), Z = sum_t u, M1 = sum_t u*x, M2 = sum_t u*x^2
  mean = M1/Z, std = sqrt(max(M2/Z - mean^2, 1e-5))
  out = LayerNorm_{3072}(concat(mean, std)) * g2 + be2

v5 engine strategy (from per-op HW microbenchmarks):
  - accumulating DVE ops are 1x (~2.24us/chunk); plain TT 2x (1.2us);
    ACT always 1x (1.96us + 0.28us accum read); Pool TT ~5.8us, no accum
  - pass1: bn_stats x4 + bn_aggr (one DVE pass -> mean AND var, 2.9us/chunk);
    a few chunks ride ACT (Copy+acc, Square+acc) while ACT is idle early
  - pass2: exp+accZ on ACT; M1 via DVE STT+acc; M2 split across DVE STT /
    Pool TT + ACT Copy+acc (3-engine balance)
  - schedule: sample-1 pass-1 interleaved into sample-0 pass-2; LN chains
    overlapped with neighbouring phases; per-sample finals
"""

import numpy as np
import ml_dtypes

B, C, T, H = 16, 1536, 2000, 128
NCORES = 8
BLOC = B // NCORES          # 2 samples per core
KC = C // 128               # 12 channel chunks
QOFF = [0, 512, 1024, 1536]  # psum quarter offsets (512/512/512/464 -> flat 0:2000)
QLEN = [512, 512, 512, 464]
EPS = 1e-5
NB = BLOC * KC              # 24 accum columns, col = b*KC + k

# --- engine-placement knobs (tuned against trace) ---
PASS1_ACT = {(0, 1), (0, 3), (0, 5), (0, 7), (0, 9), (0, 10),
             (1, 1), (1, 5), (1, 9)}
M2_POOL = set()             # pool TT causes SBUF contention: net negative
M2_SPLIT = ({(0, k) for k in range(8)} |
            {(1, k) for k in range(8)})   # DVE TT + ACT Copy+acc
M1_SPLIT = set()            # M1 as DVE TT + ACT Copy+acc
LN_HALVES = [(0, 1024), (1024, T)]  # LN chain column-split for latency

_compiled = {}


# ---------------------------------------------------------------------------
# Workaround for walrus codegen 'Too many sync wait commands': this container's
# walrus supports only ONE sync-wait slot per instruction, but Tile's wait
# assignment can attach several.  Post-pass: move excess waits onto standalone
# InstNoOp carriers spliced immediately before the instruction on the same
# engine (same-engine program order makes this equivalent).
# ---------------------------------------------------------------------------

def _apply_tile_patch():
    import concourse.mybir as mybir
    import concourse.tile as tile
    from concourse.vector_clock import ScopedClock

    if getattr(tile.TileContext, "_wait_split_patched", False):
        return

    MAX_WAITS = 1

    def split_excess_waits(nc):
        for fn in nc.m.functions:
            for bb in fn.blocks:
                il = bb.instructions
                out = []
                changed = False
                for inst in il:
                    si = getattr(inst, "sync_info", None)
                    waits = list(si.on_wait) if si is not None else []
                    if len(waits) > MAX_WAITS:
                        for j, w in enumerate(waits[MAX_WAITS:]):
                            nop = mybir.InstNoOp(
                                name=f"{inst.name}-wsplit{j}",
                                sync_info=mybir.SyncInfo(on_wait=[w], on_update=[]),
                                bass_nofuse=True,
                                engine=inst.engine,
                            )
                            nc.register_instruction(nop, overwrite=True)
                            out.append(nop)
                        si.on_wait = waits[:MAX_WAITS]
                        changed = True
                    out.append(inst)
                if changed:
                    bb.instructions = out

    def _patched_drain_and_barrier(self, tick_clock, wait_clock):
        nc = self.nc
        drain_inst = nc.sync.drain()
        wait_clock.add_sem_waits(
            drain_inst.ins, ScopedClock({None: tick_clock.global_clock})
        )
        nc.all_engine_barrier()
        assert self.sems is not None
        popped = nc._tile_sem_poison_stack.pop()
        assert popped is self._sem_poison
        nc.clear_and_free_semaphores(list(self.sems.allocated().values()))
        nc.all_engine_barrier()
        split_excess_waits(nc)

    tile.TileContext._drain_and_barrier = _patched_drain_and_barrier
    tile.TileContext._wait_split_patched = True


# ---------------------------------------------------------------------------
# Device kernel builder (one NeuronCore, BLOC samples)
# ---------------------------------------------------------------------------

def _build():
    import concourse.bass as bass
    import concourse.tile as tile
    import concourse.mybir as mybir
    from contextlib import ExitStack

    _apply_tile_patch()

    f32 = mybir.dt.float32
    bf16 = mybir.dt.bfloat16
    AL = mybir.AluOpType
    AF = mybir.ActivationFunctionType

    nc = bass.Bass(name="attnpool")

    xd = nc.dram_tensor("x", [BLOC, KC, 128, T], bf16, kind="ExternalInput")
    wad = nc.dram_tensor("wa", [128, KC, 128], bf16, kind="ExternalInput")
    wbcd = nc.dram_tensor("wbc", [128, 2 * KC, 128], bf16, kind="ExternalInput")
    w2td = nc.dram_tensor("w2t", [128, KC, 128], bf16, kind="ExternalInput")
    onesHd = nc.dram_tensor("onesH", [128, 128], bf16, kind="ExternalInput")
    onesfd = nc.dram_tensor("ones_f", [128, 128], f32, kind="ExternalInput")
    b1d = nc.dram_tensor("b1v", [128, 1], f32, kind="ExternalInput")
    g1d = nc.dram_tensor("g1v", [128, 1], f32, kind="ExternalInput")
    be1d = nc.dram_tensor("be1v", [128, 1], f32, kind="ExternalInput")
    g2d = nc.dram_tensor("g2v", [128, 2 * KC], f32, kind="ExternalInput")
    be2d = nc.dram_tensor("be2v", [128, 2 * KC], f32, kind="ExternalInput")
    yd = nc.dram_tensor("y", [BLOC, 128, 2 * KC], f32, kind="ExternalOutput")

    with tile.TileContext(nc) as tc, ExitStack() as ctx:
        singles = ctx.enter_context(tc.tile_pool(name="singles", bufs=1))
        xpool = ctx.enter_context(tc.tile_pool(name="xcache", bufs=1))
        work = ctx.enter_context(tc.tile_pool(name="work", bufs=1))
        dscr = ctx.enter_context(tc.tile_pool(name="dscr", bufs=3))
        bnp = ctx.enter_context(tc.tile_pool(name="bnp", bufs=2))
        pA = ctx.enter_context(tc.tile_pool(name="pA", bufs=1, space="PSUM"))

        # ---- small early-needed weights first (wa gates m1), then x loads ----
        wa_sb = singles.tile([128, KC, 128], bf16)
        nc.sync.dma_start(out=wa_sb, in_=wad[:, :, :])
        onesH_sb = singles.tile([128, 128], bf16)          # value 1/H
        nc.sync.dma_start(out=onesH_sb, in_=onesHd[:, :])
        b1_sb = singles.tile([128, 1], f32)
        nc.sync.dma_start(out=b1_sb, in_=b1d[:, :])
        g1_sb = singles.tile([128, 1], f32)
        nc.sync.dma_start(out=g1_sb, in_=g1d[:, :])
        be1_sb = singles.tile([128, 1], f32)
        nc.sync.dma_start(out=be1_sb, in_=be1d[:, :])

        # ---- remaining weights / constants (before the big x stream so the
        # cH matmul isn't gated on a DMA stuck behind 12 MB of x) ----
        wbc_sb = singles.tile([128, 2 * KC, 128], bf16)
        nc.sync.dma_start(out=wbc_sb, in_=wbcd[:, :, :])
        w2t_sb = singles.tile([128, KC, 128], bf16)
        nc.sync.dma_start(out=w2t_sb, in_=w2td[:, :, :])
        onesf_sb = singles.tile([128, 128], f32)
        nc.sync.dma_start(out=onesf_sb, in_=onesfd[:, :])
        g2_sb = singles.tile([128, 2 * KC], f32)
        nc.sync.dma_start(out=g2_sb, in_=g2d[:, :])
        be2_sb = singles.tile([128, 2 * KC], f32)
        nc.sync.dma_start(out=be2_sb, in_=be2d[:, :])

        x_bf = xpool.tile([128, BLOC, KC, T], bf16)         # 96 KB/part
        for b in range(BLOC):
            for k in range(KC):
                nc.sync.dma_start(out=x_bf[:, b, k, :], in_=xd[b, k, :, :])

        eps_sb = singles.tile([128, 1], f32)
        nc.vector.memset(eps_sb, EPS)

        # ---- persistent SBUF state ----
        h_bf = work.tile([128, BLOC, T], bf16)              # attention hidden
        sumx = work.tile([128, NB], f32)                    # ACT-path pass1 sums
        sumx2 = work.tile([128, NB], f32)
        bnag = work.tile([128, NB, 2], f32)                 # (mean, var) per chunk
        accZ = work.tile([128, NB], f32)
        accM1 = work.tile([128, NB], f32)
        accM2 = work.tile([128, NB], f32)
        mv_bf = work.tile([128, BLOC, 2 * KC], bf16)        # [mean0 | std0] bf16
        biasv = work.tile([128, BLOC], f32)
        # LN scratch (reused across samples)
        r_bf = work.tile([128, T], bf16)
        d_bf = work.tile([128, T], bf16)
        d2_bf = work.tile([128, T], bf16)
        rs_bf = work.tile([128, T], bf16)
        # reduction dump buffers (one per engine to avoid cross-engine WAW)
        sdump_a = work.tile([128, T], bf16)
        # pass1 finalize scratch
        msq = work.tile([128, NB], f32)
        var0 = work.tile([128, NB], f32)
        std0f = work.tile([128, NB], f32)

        def emit_pass1_chunk(b, k, m1ps):
            col = b * KC + k
            xc = x_bf[:, b, k, :]
            for q in range(4):
                o, ln = QOFF[q], QLEN[q]
                nc.tensor.matmul(
                    m1ps[:, o:o + ln], wa_sb[:, k, :],
                    x_bf[:, b, k, o:o + ln],
                    start=(k == 0), stop=(k == KC - 1),
                )
            if (b, k) in PASS1_ACT:
                # ACT path: sum(x) and sum(x^2), tiny DVE converts to mean/var
                nc.scalar.activation(
                    out=sdump_a, in_=xc, func=AF.Copy,
                    accum_out=sumx[:, col:col + 1],
                )
                nc.scalar.activation(
                    out=sdump_a, in_=xc, func=AF.Square,
                    accum_out=sumx2[:, col:col + 1],
                )
                nc.vector.tensor_scalar(
                    out=bnag[:, col, 0:1], in0=sumx[:, col:col + 1],
                    scalar1=1.0 / T, scalar2=None, op0=AL.mult,
                )
                nc.vector.tensor_mul(
                    out=msq[:, col:col + 1], in0=bnag[:, col, 0:1],
                    in1=bnag[:, col, 0:1],
                )
                nc.vector.scalar_tensor_tensor(
                    out=bnag[:, col, 1:2], in0=sumx2[:, col:col + 1],
                    scalar=1.0 / T, in1=msq[:, col:col + 1],
                    op0=AL.mult, op1=AL.subtract,
                )
            else:
                # DVE path: bn_stats gives mean AND var in one pass
                bns = bnp.tile([128, 4, 6], f32, tag="bns", name=f"bns{col}")
                for q in range(4):
                    nc.vector.bn_stats(
                        out=bns[:, q, :], in_=xc[:, 500 * q:500 * (q + 1)]
                    )
                nc.vector.bn_aggr(out=bnag[:, col, :], in_=bns)

        def emit_bn_finalize(b):
            bsl = slice(b * KC, (b + 1) * KC)
            # std0 = sqrt(max(var0, eps)); mean/std -> bf16 [mean0 | std0]
            nc.vector.tensor_scalar_max(
                out=var0[:, bsl], in0=bnag[:, bsl, 1], scalar1=EPS,
            )
            nc.scalar.activation(out=std0f[:, bsl], in_=var0[:, bsl], func=AF.Ln)
            nc.scalar.activation(out=std0f[:, bsl], in_=std0f[:, bsl],
                                 func=AF.Exp, scale=0.5)
            nc.vector.tensor_copy(out=mv_bf[:, b, 0:KC], in_=bnag[:, bsl, 0])
            nc.vector.tensor_copy(out=mv_bf[:, b, KC:2 * KC], in_=std0f[:, bsl])

        def emit_ch(b, tag):
            chps = pA.tile([128, 1], f32, tag=tag, name=f"ch{b}")
            for j in range(2 * KC):
                nc.tensor.matmul(
                    chps, wbc_sb[:, j, :], mv_bf[:, b, j:j + 1],
                    start=(j == 0), stop=(j == 2 * KC - 1),
                )
            nc.vector.tensor_add(out=biasv[:, b:b + 1], in0=chps, in1=b1_sb)

        def emit_ln(b, m1ps, tag):
            """r = relu(m1+bias); h = tanh(g1*(r-mu)*rsqrt(var+eps)+be1).

            Pipelined over two column-halves to halve time-to-first-h: each
            half runs relu -> mu-sums -> d -> d^2 -> var-sums -> ln -> exp ->
            d*rs -> tanh, with half 2 trailing half 1 by one stage.  One psum
            tile serves both mu and var: the var column-sums overwrite the mu
            columns after d consumed them (subtile WAR sync)."""
            muvar = pA.tile([128, 2048], f32, tag=tag, name=f"muvar{b}")
            mups = varps = muvar

            def stage(fn):
                for lo, hi in LN_HALVES:
                    fn(lo, hi)

            def s_relu(lo, hi):
                nc.scalar.activation(
                    out=r_bf[:, lo:hi], in_=m1ps[:, lo:hi], func=AF.Relu,
                    bias=biasv[:, b:b + 1], scale=1.0,
                )
                for q in range(4):
                    o, ln = QOFF[q], QLEN[q]
                    if o < lo or o >= hi:
                        continue
                    nc.tensor.matmul(
                        mups[:, o:o + ln], onesH_sb, r_bf[:, o:o + ln],
                        start=True, stop=True,
                    )

            def s_d(lo, hi):
                nc.vector.scalar_tensor_tensor(
                    out=d_bf[:, lo:hi], in0=r_bf[:, lo:hi], scalar=1.0,
                    in1=mups[:, lo:hi], op0=AL.mult, op1=AL.subtract,
                )
                nc.vector.tensor_mul(
                    out=d2_bf[:, lo:hi], in0=d_bf[:, lo:hi], in1=d_bf[:, lo:hi]
                )
                for q in range(4):
                    o, ln = QOFF[q], QLEN[q]
                    if o < lo or o >= hi:
                        continue
                    nc.tensor.matmul(
                        varps[:, o:o + ln], onesH_sb, d2_bf[:, o:o + ln],
                        start=True, stop=True,
                    )

            def s_rs(lo, hi):
                nc.scalar.activation(
                    out=rs_bf[:, lo:hi], in_=varps[:, lo:hi], func=AF.Ln,
                    bias=eps_sb, scale=1.0,
                )
                nc.scalar.activation(
                    out=rs_bf[:, lo:hi], in_=rs_bf[:, lo:hi], func=AF.Exp,
                    scale=-0.5,
                )

            def s_h(lo, hi):
                nc.vector.tensor_mul(
                    out=d_bf[:, lo:hi], in0=d_bf[:, lo:hi], in1=rs_bf[:, lo:hi]
                )
                nc.scalar.activation(
                    out=h_bf[:, b, lo:hi], in_=d_bf[:, lo:hi], func=AF.Tanh,
                    bias=be1_sb, scale=g1_sb,
                )

            stage(s_relu)
            stage(s_d)
            stage(s_rs)
            stage(s_h)

        def emit_pass2_chunk(b, k, tag):
            col = b * KC + k
            xc = x_bf[:, b, k, :]
            zps = pA.tile([128, 2048], f32, tag=tag, name=f"z{b}_{k}")
            for q in range(4):
                o, ln = QOFF[q], QLEN[q]
                nc.tensor.matmul(
                    zps[:, o:o + ln], w2t_sb[:, k, :],
                    h_bf[:, b, o:o + ln],
                    start=True, stop=True,
                )
            u_bf = dscr.tile([128, T], bf16, tag="u")
            nc.scalar.activation(
                out=u_bf, in_=zps[:, 0:T], func=AF.Exp,
                accum_out=accZ[:, col:col + 1],
            )
            p_bf = dscr.tile([128, T], bf16, tag="p")
            if (b, k) in M1_SPLIT:
                nc.vector.tensor_mul(out=p_bf, in0=u_bf, in1=xc)
                nc.scalar.activation(
                    out=sdump_a, in_=p_bf, func=AF.Copy,
                    accum_out=accM1[:, col:col + 1],
                )
            else:
                nc.vector.scalar_tensor_tensor(
                    out=p_bf, in0=u_bf, scalar=1.0, in1=xc,
                    op0=AL.mult, op1=AL.mult,
                    accum_out=accM1[:, col:col + 1],
                )
            q_bf = dscr.tile([128, T], bf16, tag="q")
            if (b, k) in M2_POOL:
                nc.gpsimd.tensor_mul(out=q_bf, in0=p_bf, in1=xc)
                nc.scalar.activation(
                    out=sdump_a, in_=q_bf, func=AF.Copy,
                    accum_out=accM2[:, col:col + 1],
                )
            elif (b, k) in M2_SPLIT:
                nc.vector.tensor_mul(out=q_bf, in0=p_bf, in1=xc)
                nc.scalar.activation(
                    out=sdump_a, in_=q_bf, func=AF.Copy,
                    accum_out=accM2[:, col:col + 1],
                )
            else:
                nc.vector.scalar_tensor_tensor(
                    out=q_bf, in0=p_bf, scalar=1.0, in1=xc,
                    op0=AL.mult, op1=AL.mult,
                    accum_out=accM2[:, col:col + 1],
                )

        def emit_final(b):
            """pooled mean/std -> LayerNorm(3072) -> DMA out, for sample b."""
            bsl = slice(b * KC, (b + 1) * KC)
            zr = work.tile([128, KC], f32, tag="zr", name="zr")
            nc.vector.reciprocal(out=zr, in_=accZ[:, bsl])
            v = work.tile([128, 2 * KC], f32, tag="vfin", name="vfin")
            nc.vector.tensor_mul(out=v[:, 0:KC], in0=accM1[:, bsl], in1=zr)
            ve2 = work.tile([128, KC], f32, tag="ve2", name="ve2")
            nc.vector.tensor_mul(out=ve2, in0=accM2[:, bsl], in1=zr)
            vmsq = work.tile([128, KC], f32, tag="vmsq", name="vmsq")
            nc.vector.tensor_mul(out=vmsq, in0=v[:, 0:KC], in1=v[:, 0:KC])
            nc.vector.tensor_sub(out=ve2, in0=ve2, in1=vmsq)
            nc.vector.tensor_scalar_max(out=ve2, in0=ve2, scalar1=EPS)
            nc.scalar.activation(out=v[:, KC:2 * KC], in_=ve2, func=AF.Ln)
            nc.scalar.activation(out=v[:, KC:2 * KC], in_=v[:, KC:2 * KC],
                                 func=AF.Exp, scale=0.5)

            v2 = work.tile([128, 2 * KC], f32, tag="v2fin", name="v2fin")
            nc.vector.tensor_mul(out=v2, in0=v, in1=v)
            svp = pA.tile([128, 2 * KC], f32, tag="A", name="sv")
            nc.tensor.matmul(svp, onesf_sb, v, start=True, stop=True)
            sv2p = pA.tile([128, 2 * KC], f32, tag="A1", name="sv2")
            nc.tensor.matmul(sv2p, onesf_sb, v2, start=True, stop=True)
            muf = work.tile([128, 1], f32, tag="muf", name="muf")
            nc.vector.tensor_reduce(
                out=muf, in_=svp, axis=mybir.AxisListType.X, op=AL.add
            )
            s2r = work.tile([128, 1], f32, tag="s2r", name="s2r")
            nc.vector.tensor_reduce(
                out=s2r, in_=sv2p, axis=mybir.AxisListType.X, op=AL.add
            )
            nc.vector.tensor_scalar_mul(out=muf, in0=muf, scalar1=1.0 / (2 * C))
            musq = work.tile([128, 1], f32, tag="musq", name="musq")
            nc.vector.tensor_mul(out=musq, in0=muf, in1=muf)
            nc.vector.scalar_tensor_tensor(
                out=s2r, in0=s2r, scalar=1.0 / (2 * C), in1=musq,
                op0=AL.mult, op1=AL.subtract,
            )
            nc.scalar.activation(
                out=s2r, in_=s2r, func=AF.Ln, bias=eps_sb, scale=1.0
            )
            nc.scalar.activation(out=s2r, in_=s2r, func=AF.Exp, scale=-0.5)
            vout = work.tile([128, 2 * KC], f32, tag="vout", name="vout")
            nc.vector.tensor_scalar(
                out=vout, in0=v, scalar1=muf, scalar2=s2r,
                op0=AL.subtract, op1=AL.mult,
            )
            nc.vector.tensor_mul(out=vout, in0=vout, in1=g2_sb)
            nc.vector.tensor_add(out=vout, in0=vout, in1=be2_sb)
            nc.sync.dma_start(out=yd[b, :, :], in_=vout)

        # ================= schedule =================
        # Two 4-bank psum regions (tags A / A1).  Tile's scheduler reorders
        # within engines by dependency, so emission order is mostly logical:
        # all of pass-1 (both samples) first, then LN(b0), then LN(b1)
        # overlapping the MID pass-2(b0) stream, finals batched at the end.
        m1ps0 = pA.tile([128, 2048], f32, tag="A", name="m1ps0")
        for k in range(KC):
            emit_pass1_chunk(0, k, m1ps0)
        emit_bn_finalize(0)
        emit_ch(0, "A1")
        m1ps1 = pA.tile([128, 2048], f32, tag="A1", name="m1ps1")
        for k in range(KC):
            emit_pass1_chunk(1, k, m1ps1)
        emit_ln(0, m1ps0, "A")
        emit_bn_finalize(1)
        emit_ch(1, "A")
        emit_ln(1, m1ps1, "A1")       # runs during MID; psum A1 free by then
        # MID: pass2(b0)
        for k in range(KC):
            emit_pass2_chunk(0, k, "A")
        # TAIL: pass2(b1), zps double-buffered across both regions
        for k in range(KC):
            emit_pass2_chunk(1, k, "A" if k % 2 == 0 else "A1")
        emit_final(0)
        emit_final(1)

    return nc


def _get_nc():
    if "nc" not in _compiled:
        _compiled["nc"] = _build()
    return _compiled["nc"]


def _prep_common(w1, b1, g1, be1, w2, g2, be2):
    bf = ml_dtypes.bfloat16
    # SBUF-layout weights (partition-major, contiguous DMA):
    # wa[c, k, h] = w1[h, 128k+c] ; wbc[c, j, h] ; w2t[h, k, c] = w2[128k+c, h]
    w1 = np.asarray(w1, np.float32)
    w1a = np.ascontiguousarray(
        w1[:, :C].T.reshape(KC, 128, H).transpose(1, 0, 2)).astype(bf)
    w1bT = w1[:, C:2 * C].T.reshape(KC, 128, H)
    w1cT = w1[:, 2 * C:].T.reshape(KC, 128, H)
    wbc = np.ascontiguousarray(
        np.concatenate([w1bT, w1cT], axis=0).transpose(1, 0, 2)
    ).astype(bf)
    w2t = np.ascontiguousarray(
        np.asarray(w2, np.float32).reshape(KC, 128, H).transpose(2, 0, 1)
    ).astype(bf)

    return {
        "wa": w1a,
        "wbc": wbc,
        "w2t": w2t,
        "onesH": np.full((128, 128), 1.0 / H, dtype=bf),
        "ones_f": np.ones((128, 128), dtype=np.float32),
        "b1v": np.asarray(b1, np.float32).reshape(128, 1),
        "g1v": np.asarray(g1, np.float32).reshape(128, 1),
        "be1v": np.asarray(be1, np.float32).reshape(128, 1),
        "g2v": np.ascontiguousarray(
            np.asarray(g2, np.float32).reshape(2 * KC, 128).T),
        "be2v": np.ascontiguousarray(np.asarray(be2, np.float32).reshape(2 * KC, 128).T),
    }


def kernel(x, mask, w1, b1, g1, be1, w2, b2, g2, be2, _trace=False, _tmpdir=None):
    from concourse.bass_utils import run_bass_kernel_spmd

    bf = ml_dtypes.bfloat16
    x = np.asarray(x, dtype=np.float32)
    common = _prep_common(w1, b1, g1, be1, w2, g2, be2)

    in_maps = []
    for i in range(NCORES):
        xi = np.ascontiguousarray(
            x[i * BLOC:(i + 1) * BLOC].reshape(BLOC, KC, 128, T)
        ).astype(bf)
        in_maps.append({"x": xi, **common})

    nc = _get_nc()
    kwargs = {}
    if _trace:
        kwargs = {"trace": True, "tmpdir": _tmpdir}
    res = run_bass_kernel_spmd(nc, in_maps, core_ids=list(range(NCORES)), **kwargs)

    out = np.empty((B, 2 * C, 1), dtype=np.float32)
    for i in range(NCORES):
        # y[b, p, k] -> channel 128k+p
        yi = res.results[i]["y"].transpose(0, 2, 1).reshape(BLOC, 2 * C)
        out[i * BLOC:(i + 1) * BLOC, :, 0] = yi
    if _trace:
        return out, res
    return out


# revision 14
# speedup vs baseline: 1.2120x; 1.0404x over previous
"""AttentiveStatsPool Trainium2 Bass kernel (v5).

Full-input contract: kernel(**inputs) takes the unsharded numpy inputs and
returns the full (B, 2C, 1) output.  Internally shards the batch (B=16)
across 8 NeuronCores (2 samples per core), weights replicated, no cross-core
communication.

Math per sample (mask is all-ones per the problem spec):
  mean0/var0 over T per channel, std0 = sqrt(max(var0, 1e-5))
  m1 = w1[:, :C] @ x            (H, T)
  cH = w1[:, C:2C] @ mean0 + w1[:, 2C:] @ std0 + b1   (H,)
  r = relu(m1 + cH)
  LN over H: h = tanh(g1 * (r - mu)*rsqrt(var+1e-5) + be1)
  z = w2 @ h                    (b2 drops out: softmax over T is shift-inv)
  u = exp(z), Z = sum_t u, M1 = sum_t u*x, M2 = sum_t u*x^2
  mean = M1/Z, std = sqrt(max(M2/Z - mean^2, 1e-5))
  out = LayerNorm_{3072}(concat(mean, std)) * g2 + be2

v5 engine strategy (from per-op HW microbenchmarks):
  - accumulating DVE ops are 1x (~2.24us/chunk); plain TT 2x (1.2us);
    ACT always 1x (1.96us + 0.28us accum read); Pool TT ~5.8us, no accum
  - pass1: bn_stats x4 + bn_aggr (one DVE pass -> mean AND var, 2.9us/chunk);
    a few chunks ride ACT (Copy+acc, Square+acc) while ACT is idle early
  - pass2: exp+accZ on ACT; M1 via DVE STT+acc; M2 split across DVE STT /
    Pool TT + ACT Copy+acc (3-engine balance)
  - schedule: sample-1 pass-1 interleaved into sample-0 pass-2; LN chains
    overlapped with neighbouring phases; per-sample finals
"""

import numpy as np
import ml_dtypes

B, C, T, H = 16, 1536, 2000, 128
NCORES = 8
BLOC = B // NCORES          # 2 samples per core
KC = C // 128               # 12 channel chunks
QOFF = [0, 512, 1024, 1536]  # psum quarter offsets (512/512/512/464 -> flat 0:2000)
QLEN = [512, 512, 512, 464]
EPS = 1e-5
NB = BLOC * KC              # 24 accum columns, col = b*KC + k

# --- engine-placement knobs (tuned against trace) ---
PASS1_ACT = {(0, 1), (0, 5), (0, 7), (0, 10), (1, 1), (1, 9)}
M2_POOL = set()             # pool TT causes SBUF contention: net negative
M2_SPLIT = ({(0, k) for k in range(8)} |
            {(1, k) for k in range(8)})   # DVE TT + ACT Copy+acc
M1_SPLIT = set()            # M1 as DVE TT + ACT Copy+acc
LN_HALVES = [(0, 1024), (1024, T)]  # LN chain column-split for latency

_compiled = {}


# ---------------------------------------------------------------------------
# Workaround for walrus codegen 'Too many sync wait commands': this container's
# walrus supports only ONE sync-wait slot per instruction, but Tile's wait
# assignment can attach several.  Post-pass: move excess waits onto standalone
# InstNoOp carriers spliced immediately before the instruction on the same
# engine (same-engine program order makes this equivalent).
# ---------------------------------------------------------------------------

def _apply_tile_patch():
    import concourse.mybir as mybir
    import concourse.tile as tile
    from concourse.vector_clock import ScopedClock

    if getattr(tile.TileContext, "_wait_split_patched", False):
        return

    MAX_WAITS = 1

    def split_excess_waits(nc):
        for fn in nc.m.functions:
            for bb in fn.blocks:
                il = bb.instructions
                out = []
                changed = False
                for inst in il:
                    si = getattr(inst, "sync_info", None)
                    waits = list(si.on_wait) if si is not None else []
                    if len(waits) > MAX_WAITS:
                        for j, w in enumerate(waits[MAX_WAITS:]):
                            nop = mybir.InstNoOp(
                                name=f"{inst.name}-wsplit{j}",
                                sync_info=mybir.SyncInfo(on_wait=[w], on_update=[]),
                                bass_nofuse=True,
                                engine=inst.engine,
                            )
                            nc.register_instruction(nop, overwrite=True)
                            out.append(nop)
                        si.on_wait = waits[:MAX_WAITS]
                        changed = True
                    out.append(inst)
                if changed:
                    bb.instructions = out

    def _patched_drain_and_barrier(self, tick_clock, wait_clock):
        nc = self.nc
        drain_inst = nc.sync.drain()
        wait_clock.add_sem_waits(
            drain_inst.ins, ScopedClock({None: tick_clock.global_clock})
        )
        nc.all_engine_barrier()
        assert self.sems is not None
        popped = nc._tile_sem_poison_stack.pop()
        assert popped is self._sem_poison
        nc.clear_and_free_semaphores(list(self.sems.allocated().values()))
        nc.all_engine_barrier()
        split_excess_waits(nc)

    tile.TileContext._drain_and_barrier = _patched_drain_and_barrier
    tile.TileContext._wait_split_patched = True


# ---------------------------------------------------------------------------
# Device kernel builder (one NeuronCore, BLOC samples)
# ---------------------------------------------------------------------------

def _build():
    import concourse.bass as bass
    import concourse.tile as tile
    import concourse.mybir as mybir
    from contextlib import ExitStack

    _apply_tile_patch()

    f32 = mybir.dt.float32
    bf16 = mybir.dt.bfloat16
    AL = mybir.AluOpType
    AF = mybir.ActivationFunctionType

    nc = bass.Bass(name="attnpool")

    xd = nc.dram_tensor("x", [BLOC, KC, 128, T], bf16, kind="ExternalInput")
    # packed weight blobs: 3 DMA issues instead of 10 (each costs ~600ns of
    # queue-issue + serializes the transfer stream ahead of x)
    # wbf slots: [wa(0:12) | onesH(12) | wbc(13:37) | w2t(37:49)]
    wbfd = nc.dram_tensor("wbf", [128, 49, 128], bf16, kind="ExternalInput")
    # wf32 cols: [b1(0) | g1(1) | be1(2) | g2(3:27) | be2(27:51) | onesf(51:179)]
    wf32d = nc.dram_tensor("wf32", [128, 179], f32, kind="ExternalInput")
    yd = nc.dram_tensor("y", [BLOC, 128, 2 * KC], f32, kind="ExternalOutput")

    with tile.TileContext(nc) as tc, ExitStack() as ctx:
        singles = ctx.enter_context(tc.tile_pool(name="singles", bufs=1))
        xpool = ctx.enter_context(tc.tile_pool(name="xcache", bufs=1))
        work = ctx.enter_context(tc.tile_pool(name="work", bufs=1))
        dscr = ctx.enter_context(tc.tile_pool(name="dscr", bufs=3))
        bnp = ctx.enter_context(tc.tile_pool(name="bnp", bufs=2))
        pA = ctx.enter_context(tc.tile_pool(name="pA", bufs=1, space="PSUM"))

        # ---- weights: 3 packed DMAs; wa+onesH+f32 first (gate m1/bn), the
        # big wbc|w2t blob slotted into the x stream where it is not yet
        # needed; x(b0) before x(b1) ----
        wbf_sb = singles.tile([128, 49, 128], bf16)
        nc.sync.dma_start(out=wbf_sb[:, 0:13, :], in_=wbfd[:, 0:13, :])
        wf32_sb = singles.tile([128, 179], f32)
        nc.sync.dma_start(out=wf32_sb, in_=wf32d[:, :])
        wa_sb = wbf_sb[:, 0:KC, :]
        onesH_sb = wbf_sb[:, KC, :]
        wbc_sb = wbf_sb[:, 13:13 + 2 * KC, :]
        w2t_sb = wbf_sb[:, 37:37 + KC, :]
        b1_sb = wf32_sb[:, 0:1]
        g1_sb = wf32_sb[:, 1:2]
        be1_sb = wf32_sb[:, 2:3]
        g2_sb = wf32_sb[:, 3:3 + 2 * KC]
        be2_sb = wf32_sb[:, 27:27 + 2 * KC]
        onesf_sb = wf32_sb[:, 51:179]

        x_bf = xpool.tile([128, BLOC, KC, T], bf16)         # 96 KB/part
        for k in range(KC):
            nc.sync.dma_start(out=x_bf[:, 0, k, :], in_=xd[0, k, :, :])
            if k == 5:
                nc.sync.dma_start(out=wbf_sb[:, 13:49, :], in_=wbfd[:, 13:49, :])
        for k in range(KC):
            nc.sync.dma_start(out=x_bf[:, 1, k, :], in_=xd[1, k, :, :])

        eps_sb = singles.tile([128, 1], f32)
        nc.vector.memset(eps_sb, EPS)

        # ---- persistent SBUF state ----
        h_bf = work.tile([128, BLOC, T], bf16)              # attention hidden
        sumx = work.tile([128, NB], f32)                    # ACT-path pass1 sums
        sumx2 = work.tile([128, NB], f32)
        bnag = work.tile([128, NB, 2], f32)                 # (mean, var) per chunk
        accZ = work.tile([128, NB], f32)
        accM1 = work.tile([128, NB], f32)
        accM2 = work.tile([128, NB], f32)
        mv_bf = work.tile([128, BLOC, 2 * KC], bf16)        # [mean0 | std0] bf16
        biasv = work.tile([128, BLOC], f32)
        # LN scratch (reused across samples)
        r_bf = work.tile([128, T], bf16)
        d_bf = work.tile([128, T], bf16)
        d2_bf = work.tile([128, T], bf16)
        rs_bf = work.tile([128, T], bf16)
        # reduction dump buffers (one per engine to avoid cross-engine WAW)
        sdump_a = work.tile([128, T], bf16)
        # pass1 finalize scratch
        msq = work.tile([128, NB], f32)
        var0 = work.tile([128, NB], f32)
        std0f = work.tile([128, NB], f32)

        def emit_pass1_chunk(b, k, m1ps):
            col = b * KC + k
            xc = x_bf[:, b, k, :]
            for q in range(4):
                o, ln = QOFF[q], QLEN[q]
                nc.tensor.matmul(
                    m1ps[:, o:o + ln], wa_sb[:, k, :],
                    x_bf[:, b, k, o:o + ln],
                    start=(k == 0), stop=(k == KC - 1),
                )
            if (b, k) in PASS1_ACT:
                # ACT path: sum(x) and sum(x^2), tiny DVE converts to mean/var
                nc.scalar.activation(
                    out=sdump_a, in_=xc, func=AF.Copy,
                    accum_out=sumx[:, col:col + 1],
                )
                nc.scalar.activation(
                    out=sdump_a, in_=xc, func=AF.Square,
                    accum_out=sumx2[:, col:col + 1],
                )
                nc.vector.tensor_scalar(
                    out=bnag[:, col, 0:1], in0=sumx[:, col:col + 1],
                    scalar1=1.0 / T, scalar2=None, op0=AL.mult,
                )
                nc.vector.tensor_mul(
                    out=msq[:, col:col + 1], in0=bnag[:, col, 0:1],
                    in1=bnag[:, col, 0:1],
                )
                nc.vector.scalar_tensor_tensor(
                    out=bnag[:, col, 1:2], in0=sumx2[:, col:col + 1],
                    scalar=1.0 / T, in1=msq[:, col:col + 1],
                    op0=AL.mult, op1=AL.subtract,
                )
            else:
                # DVE path: bn_stats gives mean AND var in one pass
                bns = bnp.tile([128, 4, 6], f32, tag="bns", name=f"bns{col}")
                for q in range(4):
                    nc.vector.bn_stats(
                        out=bns[:, q, :], in_=xc[:, 500 * q:500 * (q + 1)]
                    )
                nc.vector.bn_aggr(out=bnag[:, col, :], in_=bns)

        def emit_bn_finalize(b):
            bsl = slice(b * KC, (b + 1) * KC)
            # std0 = sqrt(max(var0, eps)); mean/std -> bf16 [mean0 | std0]
            nc.vector.tensor_scalar_max(
                out=var0[:, bsl], in0=bnag[:, bsl, 1], scalar1=EPS,
            )
            nc.scalar.activation(out=std0f[:, bsl], in_=var0[:, bsl], func=AF.Ln)
            nc.scalar.activation(out=std0f[:, bsl], in_=std0f[:, bsl],
                                 func=AF.Exp, scale=0.5)
            nc.vector.tensor_copy(out=mv_bf[:, b, 0:KC], in_=bnag[:, bsl, 0])
            nc.vector.tensor_copy(out=mv_bf[:, b, KC:2 * KC], in_=std0f[:, bsl])

        def emit_ch(b, tag):
            chps = pA.tile([128, 1], f32, tag=tag, name=f"ch{b}")
            for j in range(2 * KC):
                nc.tensor.matmul(
                    chps, wbc_sb[:, j, :], mv_bf[:, b, j:j + 1],
                    start=(j == 0), stop=(j == 2 * KC - 1),
                )
            nc.vector.tensor_add(out=biasv[:, b:b + 1], in0=chps, in1=b1_sb)

        def emit_ln(b, m1ps, tag):
            """r = relu(m1+bias); h = tanh(g1*(r-mu)*rsqrt(var+eps)+be1).

            Pipelined over two column-halves to halve time-to-first-h: each
            half runs relu -> mu-sums -> d -> d^2 -> var-sums -> ln -> exp ->
            d*rs -> tanh, with half 2 trailing half 1 by one stage.  One psum
            tile serves both mu and var: the var column-sums overwrite the mu
            columns after d consumed them (subtile WAR sync)."""
            muvar = pA.tile([128, 2048], f32, tag=tag, name=f"muvar{b}")
            mups = varps = muvar

            def stage(fn):
                for lo, hi in LN_HALVES:
                    fn(lo, hi)

            def s_relu(lo, hi):
                nc.scalar.activation(
                    out=r_bf[:, lo:hi], in_=m1ps[:, lo:hi], func=AF.Relu,
                    bias=biasv[:, b:b + 1], scale=1.0,
                )
                for q in range(4):
                    o, ln = QOFF[q], QLEN[q]
                    if o < lo or o >= hi:
                        continue
                    nc.tensor.matmul(
                        mups[:, o:o + ln], onesH_sb, r_bf[:, o:o + ln],
                        start=True, stop=True,
                    )

            def s_d(lo, hi):
                nc.vector.scalar_tensor_tensor(
                    out=d_bf[:, lo:hi], in0=r_bf[:, lo:hi], scalar=1.0,
                    in1=mups[:, lo:hi], op0=AL.mult, op1=AL.subtract,
                )
                nc.vector.tensor_mul(
                    out=d2_bf[:, lo:hi], in0=d_bf[:, lo:hi], in1=d_bf[:, lo:hi]
                )

            def s_var(lo, hi):
                for q in range(4):
                    o, ln = QOFF[q], QLEN[q]
                    if o < lo or o >= hi:
                        continue
                    nc.tensor.matmul(
                        varps[:, o:o + ln], onesH_sb, d2_bf[:, o:o + ln],
                        start=True, stop=True,
                    )

            def s_rs(lo, hi):
                nc.scalar.activation(
                    out=rs_bf[:, lo:hi], in_=varps[:, lo:hi], func=AF.Ln,
                    bias=eps_sb, scale=1.0,
                )
                nc.scalar.activation(
                    out=rs_bf[:, lo:hi], in_=rs_bf[:, lo:hi], func=AF.Exp,
                    scale=-0.5,
                )

            def s_h(lo, hi):
                nc.vector.tensor_mul(
                    out=d_bf[:, lo:hi], in0=d_bf[:, lo:hi], in1=rs_bf[:, lo:hi]
                )
                nc.scalar.activation(
                    out=h_bf[:, b, lo:hi], in_=d_bf[:, lo:hi], func=AF.Tanh,
                    bias=be1_sb, scale=g1_sb,
                )

            stage(s_relu)
            stage(s_d)
            stage(s_var)
            stage(s_rs)
            stage(s_h)

        def emit_pass2_chunk(b, k, tag):
            col = b * KC + k
            xc = x_bf[:, b, k, :]
            zps = pA.tile([128, 2048], f32, tag=tag, name=f"z{b}_{k}")
            for q in range(4):
                o, ln = QOFF[q], QLEN[q]
                nc.tensor.matmul(
                    zps[:, o:o + ln], w2t_sb[:, k, :],
                    h_bf[:, b, o:o + ln],
                    start=True, stop=True,
                )
            u_bf = dscr.tile([128, T], bf16, tag="u")
            nc.scalar.activation(
                out=u_bf, in_=zps[:, 0:T], func=AF.Exp,
                accum_out=accZ[:, col:col + 1],
            )
            p_bf = dscr.tile([128, T], bf16, tag="p")
            if (b, k) in M1_SPLIT:
                nc.vector.tensor_mul(out=p_bf, in0=u_bf, in1=xc)
                nc.scalar.activation(
                    out=sdump_a, in_=p_bf, func=AF.Copy,
                    accum_out=accM1[:, col:col + 1],
                )
            else:
                nc.vector.scalar_tensor_tensor(
                    out=p_bf, in0=u_bf, scalar=1.0, in1=xc,
                    op0=AL.mult, op1=AL.mult,
                    accum_out=accM1[:, col:col + 1],
                )
            q_bf = dscr.tile([128, T], bf16, tag="q")
            if (b, k) in M2_POOL:
                nc.gpsimd.tensor_mul(out=q_bf, in0=p_bf, in1=xc)
                nc.scalar.activation(
                    out=sdump_a, in_=q_bf, func=AF.Copy,
                    accum_out=accM2[:, col:col + 1],
                )
            elif (b, k) in M2_SPLIT:
                nc.vector.tensor_mul(out=q_bf, in0=p_bf, in1=xc)
                nc.scalar.activation(
                    out=sdump_a, in_=q_bf, func=AF.Copy,
                    accum_out=accM2[:, col:col + 1],
                )
            else:
                nc.vector.scalar_tensor_tensor(
                    out=q_bf, in0=p_bf, scalar=1.0, in1=xc,
                    op0=AL.mult, op1=AL.mult,
                    accum_out=accM2[:, col:col + 1],
                )

        def emit_final(b):
            """pooled mean/std -> LayerNorm(3072) -> DMA out, for sample b."""
            bsl = slice(b * KC, (b + 1) * KC)
            zr = work.tile([128, KC], f32, tag="zr", name="zr")
            nc.vector.reciprocal(out=zr, in_=accZ[:, bsl])
            v = work.tile([128, 2 * KC], f32, tag="vfin", name="vfin")
            nc.vector.tensor_mul(out=v[:, 0:KC], in0=accM1[:, bsl], in1=zr)
            ve2 = work.tile([128, KC], f32, tag="ve2", name="ve2")
            nc.vector.tensor_mul(out=ve2, in0=accM2[:, bsl], in1=zr)
            vmsq = work.tile([128, KC], f32, tag="vmsq", name="vmsq")
            nc.vector.tensor_mul(out=vmsq, in0=v[:, 0:KC], in1=v[:, 0:KC])
            nc.vector.tensor_sub(out=ve2, in0=ve2, in1=vmsq)
            nc.vector.tensor_scalar_max(out=ve2, in0=ve2, scalar1=EPS)
            nc.scalar.activation(out=v[:, KC:2 * KC], in_=ve2, func=AF.Ln)
            nc.scalar.activation(out=v[:, KC:2 * KC], in_=v[:, KC:2 * KC],
                                 func=AF.Exp, scale=0.5)

            v2 = work.tile([128, 2 * KC], f32, tag="v2fin", name="v2fin")
            nc.vector.tensor_mul(out=v2, in0=v, in1=v)
            svp = pA.tile([128, 2 * KC], f32, tag="A", name="sv")
            nc.tensor.matmul(svp, onesf_sb, v, start=True, stop=True)
            sv2p = pA.tile([128, 2 * KC], f32, tag="A1", name="sv2")
            nc.tensor.matmul(sv2p, onesf_sb, v2, start=True, stop=True)
            muf = work.tile([128, 1], f32, tag="muf", name="muf")
            nc.vector.tensor_reduce(
                out=muf, in_=svp, axis=mybir.AxisListType.X, op=AL.add
            )
            s2r = work.tile([128, 1], f32, tag="s2r", name="s2r")
            nc.vector.tensor_reduce(
                out=s2r, in_=sv2p, axis=mybir.AxisListType.X, op=AL.add
            )
            nc.vector.tensor_scalar_mul(out=muf, in0=muf, scalar1=1.0 / (2 * C))
            musq = work.tile([128, 1], f32, tag="musq", name="musq")
            nc.vector.tensor_mul(out=musq, in0=muf, in1=muf)
            nc.vector.scalar_tensor_tensor(
                out=s2r, in0=s2r, scalar=1.0 / (2 * C), in1=musq,
                op0=AL.mult, op1=AL.subtract,
            )
            nc.scalar.activation(
                out=s2r, in_=s2r, func=AF.Ln, bias=eps_sb, scale=1.0
            )
            nc.scalar.activation(out=s2r, in_=s2r, func=AF.Exp, scale=-0.5)
            vout = work.tile([128, 2 * KC], f32, tag="vout", name="vout")
            nc.vector.tensor_scalar(
                out=vout, in0=v, scalar1=muf, scalar2=s2r,
                op0=AL.subtract, op1=AL.mult,
            )
            nc.vector.tensor_mul(out=vout, in0=vout, in1=g2_sb)
            nc.vector.tensor_add(out=vout, in0=vout, in1=be2_sb)
            nc.sync.dma_start(out=yd[b, :, :], in_=vout)

        # ================= schedule =================
        # Two 4-bank psum regions (tags A / A1).  Tile's scheduler reorders
        # within engines by dependency, so emission order is mostly logical:
        # all of pass-1 (both samples) first, then LN(b0), then LN(b1)
        # overlapping the MID pass-2(b0) stream, finals batched at the end.
        m1ps0 = pA.tile([128, 2048], f32, tag="A", name="m1ps0")
        for k in range(KC):
            emit_pass1_chunk(0, k, m1ps0)
        emit_bn_finalize(0)
        emit_ch(0, "A1")
        emit_ln(0, m1ps0, "A")
        m1ps1 = pA.tile([128, 2048], f32, tag="A1", name="m1ps1")
        for k in range(KC):
            emit_pass1_chunk(1, k, m1ps1)
        emit_bn_finalize(1)
        emit_ch(1, "A")
        emit_ln(1, m1ps1, "A1")       # runs during MID; psum A1 free by then
        # MID: pass2(b0)
        for k in range(KC):
            emit_pass2_chunk(0, k, "A")
        # TAIL: pass2(b1), zps double-buffered across both regions
        emit_pass2_chunk(1, 0, "A")
        emit_final(0)
        for k in range(1, KC):
            emit_pass2_chunk(1, k, "A" if k % 2 == 0 else "A1")
        emit_final(1)

    return nc


def _get_nc():
    if "nc" not in _compiled:
        _compiled["nc"] = _build()
    return _compiled["nc"]


def _prep_common(w1, b1, g1, be1, w2, g2, be2):
    bf = ml_dtypes.bfloat16
    # SBUF-layout weights (partition-major, contiguous DMA):
    # wa[c, k, h] = w1[h, 128k+c] ; wbc[c, j, h] ; w2t[h, k, c] = w2[128k+c, h]
    w1 = np.asarray(w1, np.float32)
    w1a = np.ascontiguousarray(
        w1[:, :C].T.reshape(KC, 128, H).transpose(1, 0, 2)).astype(bf)
    w1bT = w1[:, C:2 * C].T.reshape(KC, 128, H)
    w1cT = w1[:, 2 * C:].T.reshape(KC, 128, H)
    wbc = np.ascontiguousarray(
        np.concatenate([w1bT, w1cT], axis=0).transpose(1, 0, 2)
    ).astype(bf)
    w2t = np.ascontiguousarray(
        np.asarray(w2, np.float32).reshape(KC, 128, H).transpose(2, 0, 1)
    ).astype(bf)

    onesH = np.full((128, 1, 128), 1.0 / H, dtype=bf)
    wbf = np.ascontiguousarray(
        np.concatenate([w1a, onesH, wbc, w2t], axis=1))      # [128, 49, 128]
    wf32 = np.ascontiguousarray(np.concatenate([
        np.asarray(b1, np.float32).reshape(128, 1),
        np.asarray(g1, np.float32).reshape(128, 1),
        np.asarray(be1, np.float32).reshape(128, 1),
        np.asarray(g2, np.float32).reshape(2 * KC, 128).T,
        np.asarray(be2, np.float32).reshape(2 * KC, 128).T,
        np.ones((128, 128), dtype=np.float32),
    ], axis=1))                                              # [128, 179]
    return {"wbf": wbf, "wf32": wf32}


def kernel(x, mask, w1, b1, g1, be1, w2, b2, g2, be2, _trace=False, _tmpdir=None):
    from concourse.bass_utils import run_bass_kernel_spmd

    bf = ml_dtypes.bfloat16
    x = np.asarray(x, dtype=np.float32)
    common = _prep_common(w1, b1, g1, be1, w2, g2, be2)

    in_maps = []
    for i in range(NCORES):
        xi = np.ascontiguousarray(
            x[i * BLOC:(i + 1) * BLOC].reshape(BLOC, KC, 128, T)
        ).astype(bf)
        in_maps.append({"x": xi, **common})

    nc = _get_nc()
    kwargs = {}
    if _trace:
        kwargs = {"trace": True, "tmpdir": _tmpdir}
    res = run_bass_kernel_spmd(nc, in_maps, core_ids=list(range(NCORES)), **kwargs)

    out = np.empty((B, 2 * C, 1), dtype=np.float32)
    for i in range(NCORES):
        # y[b, p, k] -> channel 128k+p
        yi = res.results[i]["y"].transpose(0, 2, 1).reshape(BLOC, 2 * C)
        out[i * BLOC:(i + 1) * BLOC, :, 0] = yi
    if _trace:
        return out, res
    return out


# revision 15
# speedup vs baseline: 1.2184x; 1.0053x over previous
"""AttentiveStatsPool Trainium2 Bass kernel (v5).

Full-input contract: kernel(**inputs) takes the unsharded numpy inputs and
returns the full (B, 2C, 1) output.  Internally shards the batch (B=16)
across 8 NeuronCores (2 samples per core), weights replicated, no cross-core
communication.

Math per sample (mask is all-ones per the problem spec):
  mean0/var0 over T per channel, std0 = sqrt(max(var0, 1e-5))
  m1 = w1[:, :C] @ x            (H, T)
  cH = w1[:, C:2C] @ mean0 + w1[:, 2C:] @ std0 + b1   (H,)
  r = relu(m1 + cH)
  LN over H: h = tanh(g1 * (r - mu)*rsqrt(var+1e-5) + be1)
  z = w2 @ h                    (b2 drops out: softmax over T is shift-inv)
  u = exp(z), Z = sum_t u, M1 = sum_t u*x, M2 = sum_t u*x^2
  mean = M1/Z, std = sqrt(max(M2/Z - mean^2, 1e-5))
  out = LayerNorm_{3072}(concat(mean, std)) * g2 + be2

v5 engine strategy (from per-op HW microbenchmarks):
  - accumulating DVE ops are 1x (~2.24us/chunk); plain TT 2x (1.2us);
    ACT always 1x (1.96us + 0.28us accum read); Pool TT ~5.8us, no accum
  - pass1: bn_stats x4 + bn_aggr (one DVE pass -> mean AND var, 2.9us/chunk);
    a few chunks ride ACT (Copy+acc, Square+acc) while ACT is idle early
  - pass2: exp+accZ on ACT; M1 via DVE STT+acc; M2 split across DVE STT /
    Pool TT + ACT Copy+acc (3-engine balance)
  - schedule: sample-1 pass-1 interleaved into sample-0 pass-2; LN chains
    overlapped with neighbouring phases; per-sample finals
"""

import numpy as np
import ml_dtypes

B, C, T, H = 16, 1536, 2000, 128
NCORES = 8
BLOC = B // NCORES          # 2 samples per core
KC = C // 128               # 12 channel chunks
QOFF = [0, 512, 1024, 1536]  # psum quarter offsets (512/512/512/464 -> flat 0:2000)
QLEN = [512, 512, 512, 464]
EPS = 1e-5
NB = BLOC * KC              # 24 accum columns, col = b*KC + k

# --- engine-placement knobs (tuned against trace) ---
PASS1_ACT = {(0, 1), (0, 5), (0, 7), (0, 10), (1, 1), (1, 9)}
M2_POOL = set()             # pool TT causes SBUF contention: net negative
M2_SPLIT = ({(0, k) for k in range(9)} |
            {(1, k) for k in range(10)})  # DVE TT + ACT Copy+acc
M1_SPLIT = set()            # M1 as DVE TT + ACT Copy+acc
LN_HALVES = [(0, 1024), (1024, T)]  # LN chain column-split for latency

_compiled = {}


# ---------------------------------------------------------------------------
# Workaround for walrus codegen 'Too many sync wait commands': this container's
# walrus supports only ONE sync-wait slot per instruction, but Tile's wait
# assignment can attach several.  Post-pass: move excess waits onto standalone
# InstNoOp carriers spliced immediately before the instruction on the same
# engine (same-engine program order makes this equivalent).
# ---------------------------------------------------------------------------

def _apply_tile_patch():
    import concourse.mybir as mybir
    import concourse.tile as tile
    from concourse.vector_clock import ScopedClock

    if getattr(tile.TileContext, "_wait_split_patched", False):
        return

    MAX_WAITS = 1

    def split_excess_waits(nc):
        for fn in nc.m.functions:
            for bb in fn.blocks:
                il = bb.instructions
                out = []
                changed = False
                for inst in il:
                    si = getattr(inst, "sync_info", None)
                    waits = list(si.on_wait) if si is not None else []
                    if len(waits) > MAX_WAITS:
                        for j, w in enumerate(waits[MAX_WAITS:]):
                            nop = mybir.InstNoOp(
                                name=f"{inst.name}-wsplit{j}",
                                sync_info=mybir.SyncInfo(on_wait=[w], on_update=[]),
                                bass_nofuse=True,
                                engine=inst.engine,
                            )
                            nc.register_instruction(nop, overwrite=True)
                            out.append(nop)
                        si.on_wait = waits[:MAX_WAITS]
                        changed = True
                    out.append(inst)
                if changed:
                    bb.instructions = out

    def _patched_drain_and_barrier(self, tick_clock, wait_clock):
        nc = self.nc
        drain_inst = nc.sync.drain()
        wait_clock.add_sem_waits(
            drain_inst.ins, ScopedClock({None: tick_clock.global_clock})
        )
        nc.all_engine_barrier()
        assert self.sems is not None
        popped = nc._tile_sem_poison_stack.pop()
        assert popped is self._sem_poison
        nc.clear_and_free_semaphores(list(self.sems.allocated().values()))
        nc.all_engine_barrier()
        split_excess_waits(nc)

    tile.TileContext._drain_and_barrier = _patched_drain_and_barrier
    tile.TileContext._wait_split_patched = True


# ---------------------------------------------------------------------------
# Device kernel builder (one NeuronCore, BLOC samples)
# ---------------------------------------------------------------------------

def _build():
    import concourse.bass as bass
    import concourse.tile as tile
    import concourse.mybir as mybir
    from contextlib import ExitStack

    _apply_tile_patch()

    f32 = mybir.dt.float32
    bf16 = mybir.dt.bfloat16
    AL = mybir.AluOpType
    AF = mybir.ActivationFunctionType

    nc = bass.Bass(name="attnpool")

    xd = nc.dram_tensor("x", [BLOC, KC, 128, T], bf16, kind="ExternalInput")
    # packed weight blobs: 3 DMA issues instead of 10 (each costs ~600ns of
    # queue-issue + serializes the transfer stream ahead of x)
    # wbf slots: [wa(0:12) | onesH(12) | wbc(13:37) | w2t(37:49)]
    wbfd = nc.dram_tensor("wbf", [128, 49, 128], bf16, kind="ExternalInput")
    # wf32 cols: [b1(0) | g1(1) | be1(2) | g2(3:27) | be2(27:51) | onesf(51:179)]
    wf32d = nc.dram_tensor("wf32", [128, 179], f32, kind="ExternalInput")
    yd = nc.dram_tensor("y", [BLOC, 128, 2 * KC], f32, kind="ExternalOutput")

    with tile.TileContext(nc) as tc, ExitStack() as ctx:
        singles = ctx.enter_context(tc.tile_pool(name="singles", bufs=1))
        xpool = ctx.enter_context(tc.tile_pool(name="xcache", bufs=1))
        work = ctx.enter_context(tc.tile_pool(name="work", bufs=1))
        dscr = ctx.enter_context(tc.tile_pool(name="dscr", bufs=3))
        bnp = ctx.enter_context(tc.tile_pool(name="bnp", bufs=2))
        pA = ctx.enter_context(tc.tile_pool(name="pA", bufs=1, space="PSUM"))

        # ---- weights: 3 packed DMAs; wa+onesH+f32 first (gate m1/bn), the
        # big wbc|w2t blob slotted into the x stream where it is not yet
        # needed; x(b0) before x(b1) ----
        wbf_sb = singles.tile([128, 49, 128], bf16)
        nc.sync.dma_start(out=wbf_sb[:, 0:13, :], in_=wbfd[:, 0:13, :])
        wf32_sb = singles.tile([128, 179], f32)
        nc.sync.dma_start(out=wf32_sb, in_=wf32d[:, :])
        wa_sb = wbf_sb[:, 0:KC, :]
        onesH_sb = wbf_sb[:, KC, :]
        wbc_sb = wbf_sb[:, 13:13 + 2 * KC, :]
        w2t_sb = wbf_sb[:, 37:37 + KC, :]
        b1_sb = wf32_sb[:, 0:1]
        g1_sb = wf32_sb[:, 1:2]
        be1_sb = wf32_sb[:, 2:3]
        g2_sb = wf32_sb[:, 3:3 + 2 * KC]
        be2_sb = wf32_sb[:, 27:27 + 2 * KC]
        onesf_sb = wf32_sb[:, 51:179]

        x_bf = xpool.tile([128, BLOC, KC, T], bf16)         # 96 KB/part
        for k in range(KC):
            nc.sync.dma_start(out=x_bf[:, 0, k, :], in_=xd[0, k, :, :])
            if k == 5:
                nc.sync.dma_start(out=wbf_sb[:, 13:49, :], in_=wbfd[:, 13:49, :])
        for k in range(KC):
            nc.sync.dma_start(out=x_bf[:, 1, k, :], in_=xd[1, k, :, :])

        eps_sb = singles.tile([128, 1], f32)
        nc.vector.memset(eps_sb, EPS)

        # ---- persistent SBUF state ----
        h_bf = work.tile([128, BLOC, T], bf16)              # attention hidden
        sumx = work.tile([128, NB], f32)                    # ACT-path pass1 sums
        sumx2 = work.tile([128, NB], f32)
        bnag = work.tile([128, NB, 2], f32)                 # (mean, var) per chunk
        accZ = work.tile([128, NB], f32)
        accM1 = work.tile([128, NB], f32)
        accM2 = work.tile([128, NB], f32)
        mv_bf = work.tile([128, BLOC, 2 * KC], bf16)        # [mean0 | std0] bf16
        biasv = work.tile([128, BLOC], f32)
        # LN scratch (reused across samples)
        r_bf = work.tile([128, T], bf16)
        d_bf = work.tile([128, T], bf16)
        d2_bf = work.tile([128, T], bf16)
        rs_bf = work.tile([128, T], bf16)
        # reduction dump buffers (one per engine to avoid cross-engine WAW)
        sdump_a = work.tile([128, T], bf16)
        # pass1 finalize scratch
        msq = work.tile([128, NB], f32)
        var0 = work.tile([128, NB], f32)
        std0f = work.tile([128, NB], f32)

        def emit_pass1_chunk(b, k, m1ps):
            col = b * KC + k
            xc = x_bf[:, b, k, :]
            for q in range(4):
                o, ln = QOFF[q], QLEN[q]
                nc.tensor.matmul(
                    m1ps[:, o:o + ln], wa_sb[:, k, :],
                    x_bf[:, b, k, o:o + ln],
                    start=(k == 0), stop=(k == KC - 1),
                )
            if (b, k) in PASS1_ACT:
                # ACT path: sum(x) and sum(x^2), tiny DVE converts to mean/var
                nc.scalar.activation(
                    out=sdump_a, in_=xc, func=AF.Copy,
                    accum_out=sumx[:, col:col + 1],
                )
                nc.scalar.activation(
                    out=sdump_a, in_=xc, func=AF.Square,
                    accum_out=sumx2[:, col:col + 1],
                )
                nc.vector.tensor_scalar(
                    out=bnag[:, col, 0:1], in0=sumx[:, col:col + 1],
                    scalar1=1.0 / T, scalar2=None, op0=AL.mult,
                )
                nc.vector.tensor_mul(
                    out=msq[:, col:col + 1], in0=bnag[:, col, 0:1],
                    in1=bnag[:, col, 0:1],
                )
                nc.vector.scalar_tensor_tensor(
                    out=bnag[:, col, 1:2], in0=sumx2[:, col:col + 1],
                    scalar=1.0 / T, in1=msq[:, col:col + 1],
                    op0=AL.mult, op1=AL.subtract,
                )
            else:
                # DVE path: bn_stats gives mean AND var in one pass
                bns = bnp.tile([128, 4, 6], f32, tag="bns", name=f"bns{col}")
                for q in range(4):
                    nc.vector.bn_stats(
                        out=bns[:, q, :], in_=xc[:, 500 * q:500 * (q + 1)]
                    )
                nc.vector.bn_aggr(out=bnag[:, col, :], in_=bns)

        def emit_bn_finalize(b):
            bsl = slice(b * KC, (b + 1) * KC)
            # std0 = sqrt(max(var0, eps)); mean/std -> bf16 [mean0 | std0]
            nc.vector.tensor_scalar_max(
                out=var0[:, bsl], in0=bnag[:, bsl, 1], scalar1=EPS,
            )
            nc.scalar.activation(out=std0f[:, bsl], in_=var0[:, bsl], func=AF.Ln)
            nc.scalar.activation(out=std0f[:, bsl], in_=std0f[:, bsl],
                                 func=AF.Exp, scale=0.5)
            nc.vector.tensor_copy(out=mv_bf[:, b, 0:KC], in_=bnag[:, bsl, 0])
            nc.vector.tensor_copy(out=mv_bf[:, b, KC:2 * KC], in_=std0f[:, bsl])

        def emit_ch(b, tag):
            chps = pA.tile([128, 1], f32, tag=tag, name=f"ch{b}")
            for j in range(2 * KC):
                nc.tensor.matmul(
                    chps, wbc_sb[:, j, :], mv_bf[:, b, j:j + 1],
                    start=(j == 0), stop=(j == 2 * KC - 1),
                )
            nc.vector.tensor_add(out=biasv[:, b:b + 1], in0=chps, in1=b1_sb)

        def emit_ln(b, m1ps, tag):
            """r = relu(m1+bias); h = tanh(g1*(r-mu)*rsqrt(var+eps)+be1).

            Pipelined over two column-halves to halve time-to-first-h: each
            half runs relu -> mu-sums -> d -> d^2 -> var-sums -> ln -> exp ->
            d*rs -> tanh, with half 2 trailing half 1 by one stage.  One psum
            tile serves both mu and var: the var column-sums overwrite the mu
            columns after d consumed them (subtile WAR sync)."""
            muvar = pA.tile([128, 2048], f32, tag=tag, name=f"muvar{b}")
            mups = varps = muvar

            def stage(fn):
                for lo, hi in LN_HALVES:
                    fn(lo, hi)

            def s_relu(lo, hi):
                nc.scalar.activation(
                    out=r_bf[:, lo:hi], in_=m1ps[:, lo:hi], func=AF.Relu,
                    bias=biasv[:, b:b + 1], scale=1.0,
                )
                for q in range(4):
                    o, ln = QOFF[q], QLEN[q]
                    if o < lo or o >= hi:
                        continue
                    nc.tensor.matmul(
                        mups[:, o:o + ln], onesH_sb, r_bf[:, o:o + ln],
                        start=True, stop=True,
                    )

            def s_d(lo, hi):
                nc.vector.scalar_tensor_tensor(
                    out=d_bf[:, lo:hi], in0=r_bf[:, lo:hi], scalar=1.0,
                    in1=mups[:, lo:hi], op0=AL.mult, op1=AL.subtract,
                )
                nc.vector.tensor_mul(
                    out=d2_bf[:, lo:hi], in0=d_bf[:, lo:hi], in1=d_bf[:, lo:hi]
                )

            def s_var(lo, hi):
                for q in range(4):
                    o, ln = QOFF[q], QLEN[q]
                    if o < lo or o >= hi:
                        continue
                    nc.tensor.matmul(
                        varps[:, o:o + ln], onesH_sb, d2_bf[:, o:o + ln],
                        start=True, stop=True,
                    )

            def s_rs(lo, hi):
                nc.scalar.activation(
                    out=rs_bf[:, lo:hi], in_=varps[:, lo:hi], func=AF.Ln,
                    bias=eps_sb, scale=1.0,
                )
                nc.scalar.activation(
                    out=rs_bf[:, lo:hi], in_=rs_bf[:, lo:hi], func=AF.Exp,
                    scale=-0.5,
                )

            def s_h(lo, hi):
                nc.vector.tensor_mul(
                    out=d_bf[:, lo:hi], in0=d_bf[:, lo:hi], in1=rs_bf[:, lo:hi]
                )
                nc.scalar.activation(
                    out=h_bf[:, b, lo:hi], in_=d_bf[:, lo:hi], func=AF.Tanh,
                    bias=be1_sb, scale=g1_sb,
                )

            stage(s_relu)
            stage(s_d)
            stage(s_var)
            stage(s_rs)
            stage(s_h)

        def emit_pass2_chunk(b, k, tag):
            col = b * KC + k
            xc = x_bf[:, b, k, :]
            zps = pA.tile([128, 2048], f32, tag=tag, name=f"z{b}_{k}")
            for q in range(4):
                o, ln = QOFF[q], QLEN[q]
                nc.tensor.matmul(
                    zps[:, o:o + ln], w2t_sb[:, k, :],
                    h_bf[:, b, o:o + ln],
                    start=True, stop=True,
                )
            u_bf = dscr.tile([128, T], bf16, tag="u")
            nc.scalar.activation(
                out=u_bf, in_=zps[:, 0:T], func=AF.Exp,
                accum_out=accZ[:, col:col + 1],
            )
            p_bf = dscr.tile([128, T], bf16, tag="p")
            if (b, k) in M1_SPLIT:
                nc.vector.tensor_mul(out=p_bf, in0=u_bf, in1=xc)
                nc.scalar.activation(
                    out=sdump_a, in_=p_bf, func=AF.Copy,
                    accum_out=accM1[:, col:col + 1],
                )
            else:
                nc.vector.scalar_tensor_tensor(
                    out=p_bf, in0=u_bf, scalar=1.0, in1=xc,
                    op0=AL.mult, op1=AL.mult,
                    accum_out=accM1[:, col:col + 1],
                )
            q_bf = dscr.tile([128, T], bf16, tag="q")
            if (b, k) in M2_POOL:
                nc.gpsimd.tensor_mul(out=q_bf, in0=p_bf, in1=xc)
                nc.scalar.activation(
                    out=sdump_a, in_=q_bf, func=AF.Copy,
                    accum_out=accM2[:, col:col + 1],
                )
            elif (b, k) in M2_SPLIT:
                nc.vector.tensor_mul(out=q_bf, in0=p_bf, in1=xc)
                nc.scalar.activation(
                    out=sdump_a, in_=q_bf, func=AF.Copy,
                    accum_out=accM2[:, col:col + 1],
                )
            else:
                nc.vector.scalar_tensor_tensor(
                    out=q_bf, in0=p_bf, scalar=1.0, in1=xc,
                    op0=AL.mult, op1=AL.mult,
                    accum_out=accM2[:, col:col + 1],
                )

        def emit_final(b):
            """pooled mean/std -> LayerNorm(3072) -> DMA out, for sample b."""
            bsl = slice(b * KC, (b + 1) * KC)
            zr = work.tile([128, KC], f32, tag="zr", name="zr")
            nc.vector.reciprocal(out=zr, in_=accZ[:, bsl])
            v = work.tile([128, 2 * KC], f32, tag="vfin", name="vfin")
            nc.vector.tensor_mul(out=v[:, 0:KC], in0=accM1[:, bsl], in1=zr)
            ve2 = work.tile([128, KC], f32, tag="ve2", name="ve2")
            nc.vector.tensor_mul(out=ve2, in0=accM2[:, bsl], in1=zr)
            vmsq = work.tile([128, KC], f32, tag="vmsq", name="vmsq")
            nc.vector.tensor_mul(out=vmsq, in0=v[:, 0:KC], in1=v[:, 0:KC])
            nc.vector.tensor_sub(out=ve2, in0=ve2, in1=vmsq)
            nc.vector.tensor_scalar_max(out=ve2, in0=ve2, scalar1=EPS)
            nc.scalar.activation(out=v[:, KC:2 * KC], in_=ve2, func=AF.Ln)
            nc.scalar.activation(out=v[:, KC:2 * KC], in_=v[:, KC:2 * KC],
                                 func=AF.Exp, scale=0.5)

            v2 = work.tile([128, 2 * KC], f32, tag="v2fin", name="v2fin")
            nc.vector.tensor_mul(out=v2, in0=v, in1=v)
            svp = pA.tile([128, 2 * KC], f32, tag="A", name="sv")
            nc.tensor.matmul(svp, onesf_sb, v, start=True, stop=True)
            sv2p = pA.tile([128, 2 * KC], f32, tag="A1", name="sv2")
            nc.tensor.matmul(sv2p, onesf_sb, v2, start=True, stop=True)
            muf = work.tile([128, 1], f32, tag="muf", name="muf")
            nc.vector.tensor_reduce(
                out=muf, in_=svp, axis=mybir.AxisListType.X, op=AL.add
            )
            s2r = work.tile([128, 1], f32, tag="s2r", name="s2r")
            nc.vector.tensor_reduce(
                out=s2r, in_=sv2p, axis=mybir.AxisListType.X, op=AL.add
            )
            nc.vector.tensor_scalar_mul(out=muf, in0=muf, scalar1=1.0 / (2 * C))
            musq = work.tile([128, 1], f32, tag="musq", name="musq")
            nc.vector.tensor_mul(out=musq, in0=muf, in1=muf)
            nc.vector.scalar_tensor_tensor(
                out=s2r, in0=s2r, scalar=1.0 / (2 * C), in1=musq,
                op0=AL.mult, op1=AL.subtract,
            )
            nc.scalar.activation(
                out=s2r, in_=s2r, func=AF.Ln, bias=eps_sb, scale=1.0
            )
            nc.scalar.activation(out=s2r, in_=s2r, func=AF.Exp, scale=-0.5)
            vout = work.tile([128, 2 * KC], f32, tag="vout", name="vout")
            nc.vector.tensor_scalar(
                out=vout, in0=v, scalar1=muf, scalar2=s2r,
                op0=AL.subtract, op1=AL.mult,
            )
            nc.vector.tensor_mul(out=vout, in0=vout, in1=g2_sb)
            nc.vector.tensor_add(out=vout, in0=vout, in1=be2_sb)
            nc.sync.dma_start(out=yd[b, :, :], in_=vout)

        # ================= schedule =================
        # Two 4-bank psum regions (tags A / A1).  Tile's scheduler reorders
        # within engines by dependency, so emission order is mostly logical:
        # all of pass-1 (both samples) first, then LN(b0), then LN(b1)
        # overlapping the MID pass-2(b0) stream, finals batched at the end.
        m1ps0 = pA.tile([128, 2048], f32, tag="A", name="m1ps0")
        for k in range(KC):
            emit_pass1_chunk(0, k, m1ps0)
        emit_bn_finalize(0)
        emit_ch(0, "A1")
        emit_ln(0, m1ps0, "A")
        m1ps1 = pA.tile([128, 2048], f32, tag="A1", name="m1ps1")
        for k in range(KC):
            emit_pass1_chunk(1, k, m1ps1)
        emit_bn_finalize(1)
        emit_ch(1, "A")
        emit_ln(1, m1ps1, "A1")       # runs during MID; psum A1 free by then
        # MID: pass2(b0); late chunks double-buffer zps via A1 (free after
        # tanh(b1)) so an ACT hiccup does not drain the z->exp->STT pipeline
        for k in range(KC):
            tag = "A" if (k < 6 or k % 2 == 1) else "A1"
            emit_pass2_chunk(0, k, tag)
        # TAIL: pass2(b1), zps double-buffered across both regions
        emit_pass2_chunk(1, 0, "A")
        emit_final(0)
        for k in range(1, KC):
            emit_pass2_chunk(1, k, "A" if k % 2 == 0 else "A1")
        emit_final(1)

    return nc


def _get_nc():
    if "nc" not in _compiled:
        _compiled["nc"] = _build()
    return _compiled["nc"]


def _prep_common(w1, b1, g1, be1, w2, g2, be2):
    bf = ml_dtypes.bfloat16
    # SBUF-layout weights (partition-major, contiguous DMA):
    # wa[c, k, h] = w1[h, 128k+c] ; wbc[c, j, h] ; w2t[h, k, c] = w2[128k+c, h]
    w1 = np.asarray(w1, np.float32)
    w1a = np.ascontiguousarray(
        w1[:, :C].T.reshape(KC, 128, H).transpose(1, 0, 2)).astype(bf)
    w1bT = w1[:, C:2 * C].T.reshape(KC, 128, H)
    w1cT = w1[:, 2 * C:].T.reshape(KC, 128, H)
    wbc = np.ascontiguousarray(
        np.concatenate([w1bT, w1cT], axis=0).transpose(1, 0, 2)
    ).astype(bf)
    w2t = np.ascontiguousarray(
        np.asarray(w2, np.float32).reshape(KC, 128, H).transpose(2, 0, 1)
    ).astype(bf)

    onesH = np.full((128, 1, 128), 1.0 / H, dtype=bf)
    wbf = np.ascontiguousarray(
        np.concatenate([w1a, onesH, wbc, w2t], axis=1))      # [128, 49, 128]
    wf32 = np.ascontiguousarray(np.concatenate([
        np.asarray(b1, np.float32).reshape(128, 1),
        np.asarray(g1, np.float32).reshape(128, 1),
        np.asarray(be1, np.float32).reshape(128, 1),
        np.asarray(g2, np.float32).reshape(2 * KC, 128).T,
        np.asarray(be2, np.float32).reshape(2 * KC, 128).T,
        np.ones((128, 128), dtype=np.float32),
    ], axis=1))                                              # [128, 179]
    return {"wbf": wbf, "wf32": wf32}


def kernel(x, mask, w1, b1, g1, be1, w2, b2, g2, be2, _trace=False, _tmpdir=None):
    from concourse.bass_utils import run_bass_kernel_spmd

    bf = ml_dtypes.bfloat16
    x = np.asarray(x, dtype=np.float32)
    common = _prep_common(w1, b1, g1, be1, w2, g2, be2)

    in_maps = []
    for i in range(NCORES):
        xi = np.ascontiguousarray(
            x[i * BLOC:(i + 1) * BLOC].reshape(BLOC, KC, 128, T)
        ).astype(bf)
        in_maps.append({"x": xi, **common})

    nc = _get_nc()
    kwargs = {}
    if _trace:
        kwargs = {"trace": True, "tmpdir": _tmpdir}
    res = run_bass_kernel_spmd(nc, in_maps, core_ids=list(range(NCORES)), **kwargs)

    out = np.empty((B, 2 * C, 1), dtype=np.float32)
    for i in range(NCORES):
        # y[b, p, k] -> channel 128k+p
        yi = res.results[i]["y"].transpose(0, 2, 1).reshape(BLOC, 2 * C)
        out[i * BLOC:(i + 1) * BLOC, :, 0] = yi
    if _trace:
        return out, res
    return out
